# revision 41
# baseline (speedup 1.0000x reference)
"""Trainium2 Bass kernel for nn_LsqNonneg: batched NNLS.

Algorithm: projected Chebyshev/heavy-ball on the V-substitution of the NNLS
KKT iteration.  With Wa_k = (1+b_k)I - a_k*AtA, the S-iteration

    S_{k+1} = relu(Wa_k S_k - b_k S_{k-1} + a_k AtX)

becomes, under V := S - Sunc with Sunc = (AtA)^-1 AtX  (E := -Sunc):

    V_{k+1} = max(Wa_k V_k - b_k V_{k-1}, E),     S = V - E

i.e. the per-iteration bias add AND the relu collapse into a single DVE
tensor_tensor(max) against the fixed threshold tile E (computed in the
prologue as (-A(AtA)^-1).T @ X -- same cost as computing AtX).

Schedule: 12 Chebyshev ramp iterations (exact [mu, L] from the host
eigendecomposition of AtA) then constant heavy-ball at the optimum
(alpha backed off 5% from the lambda=L stability edge).  Chebyshev's
transient reaches the same error ~12 iterations earlier than constant
momentum from the warm start V_1 = relu(E).

Precision: phase 1 (24 updates) runs bf16 weights+states -- its ~1e-2
bf16-state-rounding floor is then crushed by phase 2: 8 fp32r polish
updates on f32 states (fp32r noise ~2^-12), contracting the bf16-phase
noise by rho^8 while the weights carry 11-bit-dithered variants.  E stays
f32 throughout (it biases the fixed point 1:1).  The phase transition is
seamless: each matmul picks the weight flavor matching its state operand's
dtype, so V_25 (bf16) and V_26 (f32) coexist inside one update.

All weight tensors are host-packed into exact SBUF layouts (contiguous
per-partition DMAs); X rides both HW DMA queues (SP+Act) right behind the
small ramp weights, and the late-phase weights trail the X chunks since
they are not needed until ~35us in.
"""

import os
import sys

import numpy as np

for _p in ("/opt/trn_rl_repo", "/root/.axon_site/_ro/trn_rl_repo"):
    if os.path.isdir(_p) and _p not in sys.path:
        sys.path.append(_p)

import ml_dtypes
from contextlib import ExitStack

import concourse.bass as bass
import concourse.bacc as bacc
import concourse.tile as tile
from concourse import mybir
from concourse.bass_utils import run_bass_kernel_spmd

M, KD, N_FULL, N_CORES = 512, 32, 32768, 8
N_RAMP = 12          # Chebyshev ramp updates (per-iteration weights)
N_BF16 = 20          # total bf16-phase updates (incl. ramp)
N_POLISH = 6         # fp32r polish updates
TOTAL_UPD = N_BF16 + N_POLISH
N_CONST = N_BF16 + 1 - N_RAMP   # constant-phase bf16 updates (k=13..21)
SD_RHO = 0.856       # heavy-ball contraction: sigma-delta discount factor
NSTR = 4             # independent column streams in the loop
ALPHA_BACKOFF = 0.95

F32 = mybir.dt.float32
F32R = mybir.dt.float32r
BF16 = mybir.dt.bfloat16

LAST_RESULTS = None  # BassKernelResults of the most recent run (for test.py)


def build_program(ns: int):
    q = ns // 4          # free extent of the packed [128, q] layout
    qs = q // NSTR       # columns per stream
    nsl = q // 512       # 512-wide prologue slices
    assert ns % 2048 == 0 and nsl >= 1 and qs % 256 == 0

    nc = bacc.Bacc("TRN2", target_bir_lowering=False)

    n_ramp_tiles = N_RAMP + (N_RAMP - 1)   # w_1..12 then bid_2..12
    n_wrest = n_ramp_tiles + (N_CONST + 1)
    # every [128,*] dma_start costs ~128 descriptors x ~25ns of HW-DGE
    # queue time regardless of size, so everything is packed into FIVE
    # big host-side tensors: X halves, wfront (w_1+ppad), wrest
    # (wramp+wconst), w32.
    x_d = nc.dram_tensor("x", [128, 8 * (ns // 2)], BF16,
                         kind="ExternalInput")
    wfront_d = nc.dram_tensor("wfront", [128, 128 + 16 * 128], BF16,
                              kind="ExternalInput")
    wrest_d = nc.dram_tensor("wrest", [128, n_wrest * 128], BF16,
                             kind="ExternalInput")
    n_w32 = TOTAL_UPD - (N_BF16 + 1)       # updates whose cur-state is f32
    w32_d = nc.dram_tensor("w32", [128, (n_w32 + 1) * 128], F32,
                           kind="ExternalInput")
    out_d = nc.dram_tensor("s_out", [KD, ns], F32, kind="ExternalOutput")

    with ExitStack() as ctx:
        tc = ctx.enter_context(tile.TileContext(nc))
        persist = ctx.enter_context(tc.tile_pool(name="persist", bufs=1))
        xpool = ctx.enter_context(tc.tile_pool(name="xstage", bufs=1))
        psum = ctx.enter_context(tc.tile_pool(name="psum", bufs=2,
                                              space="PSUM"))

        # sync: wfront (warm-up + prologue lhs + w_1), then X half 1;
        # scalar: X half 2 immediately, then the trailing weights.
        wf_sb = persist.tile([128, 128 + 16 * 128], BF16)
        nc.sync.dma_start(wf_sb[:], wfront_d[:])
        xh = ns // 2
        # one X tile, two unequal-split DMAs so both queue fronts carry
        # ~2.27MB (wfront rides sync): the slower queue gates E
        xsplit = 4 * xh + 768
        x_sb = xpool.tile([128, 8 * xh], BF16, name="x")
        nc.scalar.dma_start(x_sb[:, 0:xsplit], x_d[:, 0:xsplit])
        nc.sync.dma_start(x_sb[:, xsplit:], x_d[:, xsplit:8 * xh])
        wrest_sb = persist.tile([128, n_wrest * 128], BF16)
        nc.scalar.dma_start(wrest_sb[:], wrest_d[:])
        w32_sb = persist.tile([128, (n_w32 + 1) * 128], F32R)
        nc.scalar.dma_start(w32_sb[:], w32_d[:].bitcast(F32R))

        def ppl(i):         # prologue lhs block i of wfront
            return wf_sb[:, 128 * (1 + i):128 * (2 + i)]

        def wr(i):          # ramp tile i (w_1 rides wfront)
            return wf_sb[:, 0:128] if i == 0 else \
                wrest_sb[:, 128 * i:128 * (i + 1)]

        def wc(i):          # constant-phase tile i
            return wrest_sb[:, 128 * (n_ramp_tiles + i):
                            128 * (n_ramp_tiles + i + 1)]

        # PE warm-up (junk matmuls on wfront) releases the PE_HAM
        # throttle during the X-DMA window.
        warm = psum.tile([128, 512], F32, name="warm", tag="pt1")
        for wi in range(24):
            nc.tensor.matmul(warm[:, 0:128], ppl(0), ppl(0),
                             start=(wi == 0), stop=(wi == 23))

        e_sb = persist.tile([128, q], F32)
        v_st = [[persist.tile([128, qs], BF16, name=f"v{i}_{j}")
                 for j in range(NSTR)] for i in range(3)]
        v32 = [[persist.tile([128, qs], F32R, name=f"v32_{i}_{j}")
                for j in range(NSTR)] for i in range(3)]
        sout = persist.tile([128, q], F32)

        # ---- prologue: E = (-A(AtA)^-1).T @ X in packed layout ----
        # scalar's X (low blocks, c=0,1) lands first: process in order
        pbs = [psum.tile([128, 512], F32, name=f"pb{s}", tag=f"pt{2 * s}")
               for s in range(nsl)]
        n_acc = [0] * nsl
        for c in (0, 1, 2, 3):
            for h in range(2):
                for g in (2 * h, 2 * h + 1):
                    lhs = ppl(4 * c + g)
                    for s in range(nsl):
                        i_acc = n_acc[s]
                        n_acc[s] = i_acc + 1
                        xoff = (2 * c + h) * xh + (g % 2) * q + 512 * s
                        nc.tensor.matmul(
                            pbs[s][:], lhs, x_sb[:, xoff:xoff + 512],
                            start=(i_acc == 0), stop=(i_acc == 15))
        spp = 512 // qs  # streams per prologue slice
        for s in range(nsl):
            nc.scalar.copy(e_sb[:, 512 * s:512 * (s + 1)], pbs[s][:])
            for jj in range(spp):
                j = spp * s + jj
                src = pbs[s][:, qs * jj:qs * (jj + 1)]
                # V1 = relu(E) straight from PSUM; Act also has the E
                # copies, so it only takes one stream
                if j == 1:
                    nc.scalar.activation(v_st[1][j][:], src,
                                         mybir.ActivationFunctionType.Relu)
                else:
                    nc.vector.tensor_scalar_max(v_st[1][j][:], src, 0.0)

        # ---- loop: update k computes V_{k+1} from V_k, V_{k-1} ----
        # state dtype: V_j is bf16 for j <= N_BF16+1, f32 after
        def vt(jgen, j):
            return (v_st if jgen <= N_BF16 + 1 else v32)[jgen % 3][j]

        for k in range(1, TOTAL_UPD + 1):
            last = (k == TOTAL_UPD)
            # cur-operand weight flavor (matches dtype of V_k)
            if k <= N_RAMP:
                wcur = wr(k - 1)
            elif k <= N_BF16 + 1:
                wcur = wc(k - (N_RAMP + 1))
            else:
                i = k - (N_BF16 + 2)
                wcur = w32_sb[:, 128 * i:128 * (i + 1)]
            # prev-operand flavor (matches dtype of V_{k-1}); k=1: b_1=0
            if k == 1:
                wprev = None
            elif k <= N_RAMP:
                wprev = wr(N_RAMP + k - 2)          # bid_k ramp tile
            elif k <= N_BF16 + 2:
                wprev = wc(N_CONST)
            else:
                wprev = w32_sb[:, 128 * n_w32:128 * (n_w32 + 1)]

            for j in range(NSTR):
                cur = vt(k, j) if k > 1 else v_st[1][j]
                prev = vt(k - 1, j) if k > 1 else None
                ptb = psum.tile([128, 512], F32, name=f"pt{k}_{j}",
                                tag=f"pt{j}")
                pt = ptb[:, 0:qs]
                esl = e_sb[:, qs * j:qs * (j + 1)]
                if k == 1:
                    nc.tensor.matmul(pt, wcur, cur[:],
                                     start=True, stop=True)
                else:
                    # -b*V_{k-1} first: never waits on the newest state
                    nc.tensor.matmul(pt, wprev, prev[:],
                                     start=True, stop=False)
                    nc.tensor.matmul(pt, wcur, cur[:],
                                     start=False, stop=True)
                if last:
                    # S = max(P,E)-E = relu(P-E): sub in PSUM, relu on Act
                    nc.vector.tensor_tensor(pt, pt, esl,
                                            op=mybir.AluOpType.subtract)
                    nc.scalar.activation(sout[:, qs * j:qs * (j + 1)], pt,
                                         mybir.ActivationFunctionType.Relu)
                else:
                    nc.vector.tensor_tensor(vt(k + 1, j)[:], pt, esl,
                                            op=mybir.AluOpType.max)

        for g in range(4):
            (nc.sync if g % 2 == 0 else nc.scalar).dma_start(
                out_d[:, g * q:(g + 1) * q], sout[32 * g:32 * (g + 1), :])

    nc.finalize()
    return nc


def _bf16(x):
    return np.asarray(x, dtype=np.float32).astype(ml_dtypes.bfloat16)


def _bf16_val(x):
    return float(np.float32(ml_dtypes.bfloat16(np.float32(x))))


def _round11(x):
    u = np.ascontiguousarray(np.asarray(x, dtype=np.float32)).view(np.uint32)
    u = ((u + np.uint32(1 << 11)) >> np.uint32(12)) << np.uint32(12)
    return u.view(np.float32)


def _dither(Mx, n, nbits):
    """n reduced-precision matrices whose per-entry mean ~= Mx."""
    M64 = np.asarray(Mx, dtype=np.float64)
    hi = (_bf16(M64) if nbits == 8 else _round11(M64)).astype(np.float64)
    ulp = 2.0 ** (np.floor(np.log2(np.abs(M64) + 1e-300)) - nbits)
    flo = np.where(hi > M64, hi - ulp, hi)
    fhi = flo + ulp
    frac = np.clip((M64 - flo) / ulp, 0, 1)
    cnt = np.rint(frac * n).astype(int)
    return [np.where(i < cnt, fhi, flo).astype(np.float32) for i in range(n)]


def _grid(M64, nbits):
    """neighboring representable values; nbits = EXPLICIT mantissa bits
    (bf16: 7, fp32r: 11) so the grid matches the storage dtype exactly."""
    hi = (_bf16(M64) if nbits == 7 else _round11(M64)).astype(np.float64)
    ulp = 2.0 ** (np.floor(np.log2(np.abs(M64) + 1e-300)) - nbits)
    flo = np.where(hi > M64, hi - ulp, hi)
    return flo, flo + ulp


def _sigma_delta(Mx, n, rho, nbits):
    """n rounded copies of Mx whose rho-discounted average is unbiased:
    per-entry error-feedback (sigma-delta) choice between the two
    neighboring representable values."""
    M64 = np.asarray(Mx, dtype=np.float64)
    flo, fhi = _grid(M64, nbits)
    D = np.zeros_like(M64)
    seq = []
    for _ in range(n):
        e_lo = rho * D + (flo - M64)
        e_hi = rho * D + (fhi - M64)
        Wq = np.where(np.abs(e_lo) <= np.abs(e_hi), flo, fhi)
        D = rho * D + (Wq - M64)
        seq.append(Wq.astype(np.float32))
    return seq


def _blockdiag(Mt, dtype):
    """lhsT tile: 4-group block-diagonal of Mt (already transposed)."""
    out = np.zeros((128, 128), dtype=dtype)
    for g in range(4):
        out[32 * g:32 * (g + 1), 32 * g:32 * (g + 1)] = Mt
    return out


def _cheby_params(L, mu, n_ramp, n_total):
    d = (L + mu) / 2.0
    cc = (L - mu) / 2.0
    al = [0.0] * (n_total + 1)
    be = [0.0] * (n_total + 1)
    w_prev = 0.0
    for k in range(1, n_total + 1):
        w = 1.0 / d if k == 1 else 1.0 / (d - cc * cc / 4.0 * w_prev)
        be[k] = (cc / 2.0) ** 2 * w_prev * w if k > 1 else 0.0
        al[k] = w
        w_prev = w
    kap = L / mu
    aH = ALPHA_BACKOFF * 4.0 / (np.sqrt(L) + np.sqrt(mu)) ** 2
    bH = ((np.sqrt(kap) - 1.0) / (np.sqrt(kap) + 1.0)) ** 2
    for k in range(n_ramp + 1, n_total + 1):
        al[k] = aH
        be[k] = bH
    return al, be


def host_prep(A: np.ndarray):
    A64 = np.asarray(A, dtype=np.float64)
    AtA = A64.T @ A64
    ev = np.linalg.eigvalsh(AtA)
    L, mu = ev[-1], ev[0]
    I = np.eye(KD)
    al, be = _cheby_params(L, mu, N_RAMP, TOTAL_UPD)

    # ramp tiles: w_1..N_RAMP (whole Wa_k, nearest-bf16), bid_2..N_RAMP
    n_ramp_tiles = N_RAMP + (N_RAMP - 1)
    wramp = np.zeros((128, n_ramp_tiles * 128), dtype=ml_dtypes.bfloat16)
    for k in range(1, N_RAMP + 1):
        bq = _bf16_val(be[k])
        Wa = (1.0 + bq) * I - al[k] * AtA
        wramp[:, 128 * (k - 1):128 * k] = _blockdiag(
            _bf16(Wa.T), ml_dtypes.bfloat16)
        if k >= 2:
            bid = np.zeros((32, 32), dtype=np.float64)
            np.fill_diagonal(bid, -bq)
            wramp[:, 128 * (N_RAMP + k - 2):128 * (N_RAMP + k - 1)] = \
                _blockdiag(_bf16(bid), ml_dtypes.bfloat16)

    # constant phase: one bf16-exact beta everywhere, sigma-delta
    # (rho-discounted per-entry error feedback) rounding sequences so the
    # effective weight seen by the fixed point is unbiased even over few
    # iterations (random dither left a ~1e-2 realization lottery).
    aH, bH = al[N_RAMP + 1], be[N_RAMP + 1]
    bHq = _bf16_val(bH)
    WaH = (1.0 + bHq) * I - aH * AtA
    n_w32 = TOTAL_UPD - (N_BF16 + 1)
    wconst = np.zeros((128, (N_CONST + 1) * 128), dtype=ml_dtypes.bfloat16)
    for i, v in enumerate(_sigma_delta(WaH.T, N_CONST, SD_RHO, 7)):
        wconst[:, 128 * i:128 * (i + 1)] = _blockdiag(v, ml_dtypes.bfloat16)
    bidH = np.diag([-bHq] * KD)
    wconst[:, 128 * N_CONST:] = _blockdiag(
        _bf16(bidH), ml_dtypes.bfloat16)

    # fp32r phase: same beta (bf16 value is f32- and fp32r-exact)
    w32 = np.zeros((128, (n_w32 + 1) * 128), dtype=np.float32)
    for i, v in enumerate(_sigma_delta(WaH.T, n_w32, SD_RHO, 11)):
        w32[:, 128 * i:128 * (i + 1)] = _blockdiag(v, np.float32)
    w32[:, 128 * n_w32:] = _blockdiag(
        np.diag([-bHq] * KD).astype(np.float32), np.float32)

    # prologue lhs: -A(AtA)^-1, bf16-rounded with per-column error
    # feedback DOWN THE ROW (contraction) AXIS: since E = Pq.T @ X and
    # X ~ U[0,1) is row-iid, keeping each column's running rounding
    # residual near zero cancels the mean-field bias of E.
    P = A64 @ np.linalg.inv(AtA)
    Pm = -P
    flo, fhi = _grid(Pm, 7)
    Pq = np.zeros_like(Pm)
    D = np.zeros(Pm.shape[1])
    for m in range(Pm.shape[0]):
        e_lo = D + (flo[m] - Pm[m])
        e_hi = D + (fhi[m] - Pm[m])
        take_lo = np.abs(e_lo) <= np.abs(e_hi)
        Pq[m] = np.where(take_lo, flo[m], fhi[m])
        D = np.where(take_lo, e_lo, e_hi)
    ppad = np.zeros((128, 16 * 128), dtype=ml_dtypes.bfloat16)
    for cch in range(4):
        pv = Pq[128 * cch:128 * (cch + 1), :]
        for g in range(4):
            ppad[:, 128 * (4 * cch + g) + 32 * g:
                 128 * (4 * cch + g) + 32 * (g + 1)] = _bf16(pv)

    # merge into the minimal-DMA layouts: wfront = [w_1 | ppad],
    # wrest = [wramp | wconst]
    wfront = np.concatenate([wramp[:, 0:128], ppad], axis=1)
    wrest = np.concatenate([wramp, wconst], axis=1)
    return wfront, wrest, w32


_PROGRAM_CACHE = {}


def _get_program(ns):
    if ns not in _PROGRAM_CACHE:
        _PROGRAM_CACHE[ns] = build_program(ns)
    return _PROGRAM_CACHE[ns]


def kernel(X: np.ndarray, A: np.ndarray) -> np.ndarray:
    global LAST_RESULTS
    X = np.ascontiguousarray(np.asarray(X, dtype=np.float32))
    A = np.ascontiguousarray(np.asarray(A, dtype=np.float32))
    assert X.shape == (M, N_FULL) and A.shape == (M, KD)

    ns = N_FULL // N_CORES
    wfront, wrest, w32 = host_prep(A)
    nc = _get_program(ns)

    Xb = _bf16(X)
    in_maps = []
    for c in range(N_CORES):
        # pack the shard so each partition line is one contiguous
        # descriptor: x[p, (2c+h)*2048 + j] = Xb[128c+p, 2048h+j]
        xs = np.asarray(Xb[:, c * ns:(c + 1) * ns])
        xp = np.ascontiguousarray(
            xs.reshape(4, 128, 2, ns // 2).transpose(1, 0, 2, 3)
            .reshape(128, 4 * ns))
        in_maps.append({
            "x": xp,
            "wfront": wfront,
            "wrest": wrest,
            "w32": w32,
        })

    res = run_bass_kernel_spmd(nc, in_maps, core_ids=list(range(N_CORES)))
    LAST_RESULTS = res
    S = np.concatenate([res.results[c]["s_out"] for c in range(N_CORES)],
                       axis=1)
    return np.ascontiguousarray(S.astype(np.float32))


# revision 43
# speedup vs baseline: 1.0468x; 1.0468x over previous
"""Trainium2 Bass kernel for nn_LsqNonneg: batched NNLS.

Algorithm: projected Chebyshev/heavy-ball on the V-substitution of the NNLS
KKT iteration.  With Wa_k = (1+b_k)I - a_k*AtA, the S-iteration

    S_{k+1} = relu(Wa_k S_k - b_k S_{k-1} + a_k AtX)

becomes, under V := S - Sunc with Sunc = (AtA)^-1 AtX  (E := -Sunc):

    V_{k+1} = max(Wa_k V_k - b_k V_{k-1}, E),     S = V - E

i.e. the per-iteration bias add AND the relu collapse into a single DVE
tensor_tensor(max) against the fixed threshold tile E (computed in the
prologue as (-A(AtA)^-1).T @ X -- same cost as computing AtX).

Schedule: 12 Chebyshev ramp iterations (exact [mu, L] from the host
eigendecomposition of AtA) then constant heavy-ball at the optimum
(alpha backed off 5% from the lambda=L stability edge).  Chebyshev's
transient reaches the same error ~12 iterations earlier than constant
momentum from the warm start V_1 = relu(E).

Precision: phase 1 (24 updates) runs bf16 weights+states -- its ~1e-2
bf16-state-rounding floor is then crushed by phase 2: 8 fp32r polish
updates on f32 states (fp32r noise ~2^-12), contracting the bf16-phase
noise by rho^8 while the weights carry 11-bit-dithered variants.  E stays
f32 throughout (it biases the fixed point 1:1).  The phase transition is
seamless: each matmul picks the weight flavor matching its state operand's
dtype, so V_25 (bf16) and V_26 (f32) coexist inside one update.

All weight tensors are host-packed into exact SBUF layouts (contiguous
per-partition DMAs); X rides both HW DMA queues (SP+Act) right behind the
small ramp weights, and the late-phase weights trail the X chunks since
they are not needed until ~35us in.
"""

import os
import sys

import numpy as np

for _p in ("/opt/trn_rl_repo", "/root/.axon_site/_ro/trn_rl_repo"):
    if os.path.isdir(_p) and _p not in sys.path:
        sys.path.append(_p)

import ml_dtypes
from contextlib import ExitStack

import concourse.bass as bass
import concourse.bacc as bacc
import concourse.tile as tile
from concourse import mybir
from concourse.bass_utils import run_bass_kernel_spmd

M, KD, N_FULL, N_CORES = 512, 32, 32768, 8
N_RAMP = 12          # Chebyshev ramp updates (per-iteration weights)
N_BF16 = 18          # total bf16-phase updates (incl. ramp)
N_POLISH = 5         # fp32r polish updates
TOTAL_UPD = N_BF16 + N_POLISH
N_CONST = N_BF16 + 1 - N_RAMP   # constant-phase bf16 updates (k=13..21)
SD_RHO = 0.856       # heavy-ball contraction: sigma-delta discount factor
NSTR = 4             # independent column streams in the loop
ALPHA_BACKOFF = 0.95

F32 = mybir.dt.float32
F32R = mybir.dt.float32r
BF16 = mybir.dt.bfloat16

LAST_RESULTS = None  # BassKernelResults of the most recent run (for test.py)


def build_program(ns: int):
    q = ns // 4          # free extent of the packed [128, q] layout
    qs = q // NSTR       # columns per stream
    nsl = q // 512       # 512-wide prologue slices
    assert ns % 2048 == 0 and nsl >= 1 and qs % 256 == 0

    nc = bacc.Bacc("TRN2", target_bir_lowering=False)

    n_ramp_tiles = N_RAMP + (N_RAMP - 1)   # w_1..12 then bid_2..12
    n_wrest = n_ramp_tiles + (N_CONST + 1)
    # every [128,*] dma_start costs ~128 descriptors x ~25ns of HW-DGE
    # queue time regardless of size, so everything is packed into FIVE
    # big host-side tensors: X halves, wfront (w_1+ppad), wrest
    # (wramp+wconst), w32.
    x_d = nc.dram_tensor("x", [128, 8 * (ns // 2)], BF16,
                         kind="ExternalInput")
    wfront_d = nc.dram_tensor("wfront", [128, 128 + 16 * 128], BF16,
                              kind="ExternalInput")
    wrest_d = nc.dram_tensor("wrest", [128, n_wrest * 128], BF16,
                             kind="ExternalInput")
    n_w32 = TOTAL_UPD - (N_BF16 + 1)       # updates whose cur-state is f32
    w32_d = nc.dram_tensor("w32", [128, (n_w32 + 1) * 128], F32,
                           kind="ExternalInput")
    out_d = nc.dram_tensor("s_out", [KD, ns], F32, kind="ExternalOutput")

    with ExitStack() as ctx:
        tc = ctx.enter_context(tile.TileContext(nc))
        persist = ctx.enter_context(tc.tile_pool(name="persist", bufs=1))
        xpool = ctx.enter_context(tc.tile_pool(name="xstage", bufs=1))
        psum = ctx.enter_context(tc.tile_pool(name="psum", bufs=2,
                                              space="PSUM"))

        # sync: wfront (warm-up + prologue lhs + w_1), then X half 1;
        # scalar: X half 2 immediately, then the trailing weights.
        wf_sb = persist.tile([128, 128 + 16 * 128], BF16)
        nc.sync.dma_start(wf_sb[:], wfront_d[:])
        xh = ns // 2
        xb_sb = xpool.tile([128, 4 * xh], BF16, name="xb")   # c=2,3
        nc.scalar.dma_start(xb_sb[:], x_d[:, 4 * xh:8 * xh])
        xa_sb = xpool.tile([128, 4 * xh], BF16, name="xa")   # c=0,1
        nc.sync.dma_start(xa_sb[:], x_d[:, 0:4 * xh])
        wrest_sb = persist.tile([128, n_wrest * 128], BF16)
        nc.scalar.dma_start(wrest_sb[:], wrest_d[:])
        w32_sb = persist.tile([128, (n_w32 + 1) * 128], F32R)
        nc.scalar.dma_start(w32_sb[:], w32_d[:].bitcast(F32R))

        def ppl(i):         # prologue lhs block i of wfront
            return wf_sb[:, 128 * (1 + i):128 * (2 + i)]

        def wr(i):          # ramp tile i (w_1 rides wfront)
            return wf_sb[:, 0:128] if i == 0 else \
                wrest_sb[:, 128 * i:128 * (i + 1)]

        def wc(i):          # constant-phase tile i
            return wrest_sb[:, 128 * (n_ramp_tiles + i):
                            128 * (n_ramp_tiles + i + 1)]

        # PE warm-up (junk matmuls on wfront) releases the PE_HAM
        # throttle during the X-DMA window.
        warm = psum.tile([128, 512], F32, name="warm", tag="pt1")
        for wi in range(24):
            nc.tensor.matmul(warm[:, 0:128], ppl(0), ppl(0),
                             start=(wi == 0), stop=(wi == 23))

        e_sb = persist.tile([128, q], F32)
        v_st = [[persist.tile([128, qs], BF16, name=f"v{i}_{j}")
                 for j in range(NSTR)] for i in range(3)]
        v32 = [[persist.tile([128, qs], F32R, name=f"v32_{i}_{j}")
                for j in range(NSTR)] for i in range(3)]
        sout = persist.tile([128, q], F32)

        # ---- prologue: E = (-A(AtA)^-1).T @ X in packed layout ----
        # scalar's X half (c=2,3) lands first, so process it first
        pbs = [psum.tile([128, 512], F32, name=f"pb{s}", tag=f"pt{2 * s}")
               for s in range(nsl)]
        n_acc = [0] * nsl
        for c in (2, 3, 0, 1):
            xt = xb_sb if c >= 2 else xa_sb
            xbase = (c - 2 if c >= 2 else c) * 2 * xh
            for h in range(2):
                for g in (2 * h, 2 * h + 1):
                    lhs = ppl(4 * c + g)
                    for s in range(nsl):
                        i_acc = n_acc[s]
                        n_acc[s] = i_acc + 1
                        xoff = xbase + h * xh + (g % 2) * q + 512 * s
                        nc.tensor.matmul(
                            pbs[s][:], lhs, xt[:, xoff:xoff + 512],
                            start=(i_acc == 0), stop=(i_acc == 15))
        spp = 512 // qs  # streams per prologue slice
        for s in range(nsl):
            nc.scalar.copy(e_sb[:, 512 * s:512 * (s + 1)], pbs[s][:])
            for jj in range(spp):
                j = spp * s + jj
                src = pbs[s][:, qs * jj:qs * (jj + 1)]
                # V1 = relu(E) straight from PSUM; Act also has the E
                # copies, so it only takes one stream
                if j == 1:
                    nc.scalar.activation(v_st[1][j][:], src,
                                         mybir.ActivationFunctionType.Relu)
                else:
                    nc.vector.tensor_scalar_max(v_st[1][j][:], src, 0.0)

        # ---- loop: update k computes V_{k+1} from V_k, V_{k-1} ----
        # state dtype: V_j is bf16 for j <= N_BF16+1, f32 after
        def vt(jgen, j):
            return (v_st if jgen <= N_BF16 + 1 else v32)[jgen % 3][j]

        for k in range(1, TOTAL_UPD + 1):
            last = (k == TOTAL_UPD)
            # cur-operand weight flavor (matches dtype of V_k)
            if k <= N_RAMP:
                wcur = wr(k - 1)
            elif k <= N_BF16 + 1:
                wcur = wc(k - (N_RAMP + 1))
            else:
                i = k - (N_BF16 + 2)
                wcur = w32_sb[:, 128 * i:128 * (i + 1)]
            # prev-operand flavor (matches dtype of V_{k-1}); k=1: b_1=0
            if k == 1:
                wprev = None
            elif k <= N_RAMP:
                wprev = wr(N_RAMP + k - 2)          # bid_k ramp tile
            elif k <= N_BF16 + 2:
                wprev = wc(N_CONST)
            else:
                wprev = w32_sb[:, 128 * n_w32:128 * (n_w32 + 1)]

            for j in range(NSTR):
                cur = vt(k, j) if k > 1 else v_st[1][j]
                prev = vt(k - 1, j) if k > 1 else None
                ptb = psum.tile([128, 512], F32, name=f"pt{k}_{j}",
                                tag=f"pt{j}")
                pt = ptb[:, 0:qs]
                esl = e_sb[:, qs * j:qs * (j + 1)]
                if k == 1:
                    nc.tensor.matmul(pt, wcur, cur[:],
                                     start=True, stop=True)
                else:
                    # -b*V_{k-1} first: never waits on the newest state
                    nc.tensor.matmul(pt, wprev, prev[:],
                                     start=True, stop=False)
                    nc.tensor.matmul(pt, wcur, cur[:],
                                     start=False, stop=True)
                if last:
                    # S = max(P,E)-E = relu(P-E): sub in PSUM, relu on Act
                    nc.vector.tensor_tensor(pt, pt, esl,
                                            op=mybir.AluOpType.subtract)
                    nc.scalar.activation(sout[:, qs * j:qs * (j + 1)], pt,
                                         mybir.ActivationFunctionType.Relu)
                else:
                    nc.vector.tensor_tensor(vt(k + 1, j)[:], pt, esl,
                                            op=mybir.AluOpType.max)

        for g in range(4):
            (nc.sync if g % 2 == 0 else nc.scalar).dma_start(
                out_d[:, g * q:(g + 1) * q], sout[32 * g:32 * (g + 1), :])

    nc.finalize()
    return nc


def _bf16(x):
    return np.asarray(x, dtype=np.float32).astype(ml_dtypes.bfloat16)


def _bf16_val(x):
    return float(np.float32(ml_dtypes.bfloat16(np.float32(x))))


def _round11(x):
    u = np.ascontiguousarray(np.asarray(x, dtype=np.float32)).view(np.uint32)
    u = ((u + np.uint32(1 << 11)) >> np.uint32(12)) << np.uint32(12)
    return u.view(np.float32)


def _dither(Mx, n, nbits):
    """n reduced-precision matrices whose per-entry mean ~= Mx."""
    M64 = np.asarray(Mx, dtype=np.float64)
    hi = (_bf16(M64) if nbits == 8 else _round11(M64)).astype(np.float64)
    ulp = 2.0 ** (np.floor(np.log2(np.abs(M64) + 1e-300)) - nbits)
    flo = np.where(hi > M64, hi - ulp, hi)
    fhi = flo + ulp
    frac = np.clip((M64 - flo) / ulp, 0, 1)
    cnt = np.rint(frac * n).astype(int)
    return [np.where(i < cnt, fhi, flo).astype(np.float32) for i in range(n)]


def _grid(M64, nbits):
    """neighboring representable values; nbits = EXPLICIT mantissa bits
    (bf16: 7, fp32r: 11) so the grid matches the storage dtype exactly."""
    hi = (_bf16(M64) if nbits == 7 else _round11(M64)).astype(np.float64)
    ulp = 2.0 ** (np.floor(np.log2(np.abs(M64) + 1e-300)) - nbits)
    flo = np.where(hi > M64, hi - ulp, hi)
    return flo, flo + ulp


def _sigma_delta(Mx, n, rho, nbits):
    """n rounded copies of Mx whose rho-discounted average is unbiased:
    per-entry error-feedback (sigma-delta) choice between the two
    neighboring representable values."""
    M64 = np.asarray(Mx, dtype=np.float64)
    flo, fhi = _grid(M64, nbits)
    D = np.zeros_like(M64)
    seq = []
    for _ in range(n):
        e_lo = rho * D + (flo - M64)
        e_hi = rho * D + (fhi - M64)
        Wq = np.where(np.abs(e_lo) <= np.abs(e_hi), flo, fhi)
        D = rho * D + (Wq - M64)
        seq.append(Wq.astype(np.float32))
    return seq


def _blockdiag(Mt, dtype):
    """lhsT tile: 4-group block-diagonal of Mt (already transposed)."""
    out = np.zeros((128, 128), dtype=dtype)
    for g in range(4):
        out[32 * g:32 * (g + 1), 32 * g:32 * (g + 1)] = Mt
    return out


def _cheby_params(L, mu, n_ramp, n_total):
    d = (L + mu) / 2.0
    cc = (L - mu) / 2.0
    al = [0.0] * (n_total + 1)
    be = [0.0] * (n_total + 1)
    w_prev = 0.0
    for k in range(1, n_total + 1):
        w = 1.0 / d if k == 1 else 1.0 / (d - cc * cc / 4.0 * w_prev)
        be[k] = (cc / 2.0) ** 2 * w_prev * w if k > 1 else 0.0
        al[k] = w
        w_prev = w
    kap = L / mu
    aH = ALPHA_BACKOFF * 4.0 / (np.sqrt(L) + np.sqrt(mu)) ** 2
    bH = ((np.sqrt(kap) - 1.0) / (np.sqrt(kap) + 1.0)) ** 2
    for k in range(n_ramp + 1, n_total + 1):
        al[k] = aH
        be[k] = bH
    return al, be


def host_prep(A: np.ndarray):
    A64 = np.asarray(A, dtype=np.float64)
    AtA = A64.T @ A64
    ev = np.linalg.eigvalsh(AtA)
    L, mu = ev[-1], ev[0]
    I = np.eye(KD)
    al, be = _cheby_params(L, mu, N_RAMP, TOTAL_UPD)

    # ramp tiles: w_1..N_RAMP (whole Wa_k, nearest-bf16), bid_2..N_RAMP
    n_ramp_tiles = N_RAMP + (N_RAMP - 1)
    wramp = np.zeros((128, n_ramp_tiles * 128), dtype=ml_dtypes.bfloat16)
    for k in range(1, N_RAMP + 1):
        bq = _bf16_val(be[k])
        Wa = (1.0 + bq) * I - al[k] * AtA
        wramp[:, 128 * (k - 1):128 * k] = _blockdiag(
            _bf16(Wa.T), ml_dtypes.bfloat16)
        if k >= 2:
            bid = np.zeros((32, 32), dtype=np.float64)
            np.fill_diagonal(bid, -bq)
            wramp[:, 128 * (N_RAMP + k - 2):128 * (N_RAMP + k - 1)] = \
                _blockdiag(_bf16(bid), ml_dtypes.bfloat16)

    # constant phase: one bf16-exact beta everywhere, sigma-delta
    # (rho-discounted per-entry error feedback) rounding sequences so the
    # effective weight seen by the fixed point is unbiased even over few
    # iterations (random dither left a ~1e-2 realization lottery).
    aH, bH = al[N_RAMP + 1], be[N_RAMP + 1]
    bHq = _bf16_val(bH)
    WaH = (1.0 + bHq) * I - aH * AtA
    n_w32 = TOTAL_UPD - (N_BF16 + 1)
    wconst = np.zeros((128, (N_CONST + 1) * 128), dtype=ml_dtypes.bfloat16)
    for i, v in enumerate(_sigma_delta(WaH.T, N_CONST, SD_RHO, 7)):
        wconst[:, 128 * i:128 * (i + 1)] = _blockdiag(v, ml_dtypes.bfloat16)
    bidH = np.diag([-bHq] * KD)
    wconst[:, 128 * N_CONST:] = _blockdiag(
        _bf16(bidH), ml_dtypes.bfloat16)

    # fp32r phase: same beta (bf16 value is f32- and fp32r-exact)
    w32 = np.zeros((128, (n_w32 + 1) * 128), dtype=np.float32)
    for i, v in enumerate(_sigma_delta(WaH.T, n_w32, SD_RHO, 11)):
        w32[:, 128 * i:128 * (i + 1)] = _blockdiag(v, np.float32)
    w32[:, 128 * n_w32:] = _blockdiag(
        np.diag([-bHq] * KD).astype(np.float32), np.float32)

    # prologue lhs: -A(AtA)^-1, bf16-rounded with per-column error
    # feedback DOWN THE ROW (contraction) AXIS: since E = Pq.T @ X and
    # X ~ U[0,1) is row-iid, keeping each column's running rounding
    # residual near zero cancels the mean-field bias of E.
    P = A64 @ np.linalg.inv(AtA)
    Pm = -P
    flo, fhi = _grid(Pm, 7)
    Pq = np.zeros_like(Pm)
    D = np.zeros(Pm.shape[1])
    for m in range(Pm.shape[0]):
        e_lo = D + (flo[m] - Pm[m])
        e_hi = D + (fhi[m] - Pm[m])
        take_lo = np.abs(e_lo) <= np.abs(e_hi)
        Pq[m] = np.where(take_lo, flo[m], fhi[m])
        D = np.where(take_lo, e_lo, e_hi)
    ppad = np.zeros((128, 16 * 128), dtype=ml_dtypes.bfloat16)
    for cch in range(4):
        pv = Pq[128 * cch:128 * (cch + 1), :]
        for g in range(4):
            ppad[:, 128 * (4 * cch + g) + 32 * g:
                 128 * (4 * cch + g) + 32 * (g + 1)] = _bf16(pv)

    # merge into the minimal-DMA layouts: wfront = [w_1 | ppad],
    # wrest = [wramp | wconst]
    wfront = np.concatenate([wramp[:, 0:128], ppad], axis=1)
    wrest = np.concatenate([wramp, wconst], axis=1)
    return wfront, wrest, w32


_PROGRAM_CACHE = {}


def _get_program(ns):
    if ns not in _PROGRAM_CACHE:
        _PROGRAM_CACHE[ns] = build_program(ns)
    return _PROGRAM_CACHE[ns]


def kernel(X: np.ndarray, A: np.ndarray) -> np.ndarray:
    global LAST_RESULTS
    X = np.ascontiguousarray(np.asarray(X, dtype=np.float32))
    A = np.ascontiguousarray(np.asarray(A, dtype=np.float32))
    assert X.shape == (M, N_FULL) and A.shape == (M, KD)

    ns = N_FULL // N_CORES
    wfront, wrest, w32 = host_prep(A)
    nc = _get_program(ns)

    Xb = _bf16(X)
    in_maps = []
    for c in range(N_CORES):
        # pack the shard so each partition line is one contiguous
        # descriptor: x[p, (2c+h)*2048 + j] = Xb[128c+p, 2048h+j]
        xs = np.asarray(Xb[:, c * ns:(c + 1) * ns])
        xp = np.ascontiguousarray(
            xs.reshape(4, 128, 2, ns // 2).transpose(1, 0, 2, 3)
            .reshape(128, 4 * ns))
        in_maps.append({
            "x": xp,
            "wfront": wfront,
            "wrest": wrest,
            "w32": w32,
        })

    res = run_bass_kernel_spmd(nc, in_maps, core_ids=list(range(N_CORES)))
    LAST_RESULTS = res
    S = np.concatenate([res.results[c]["s_out"] for c in range(N_CORES)],
                       axis=1)
    return np.ascontiguousarray(S.astype(np.float32))


# revision 46
# speedup vs baseline: 1.1611x; 1.1092x over previous
"""Trainium2 Bass kernel for nn_LsqNonneg: batched NNLS.

Algorithm: projected Chebyshev/heavy-ball on the V-substitution of the NNLS
KKT iteration.  With Wa_k = (1+b_k)I - a_k*AtA, the S-iteration

    S_{k+1} = relu(Wa_k S_k - b_k S_{k-1} + a_k AtX)

becomes, under V := S - Sunc with Sunc = (AtA)^-1 AtX  (E := -Sunc):

    V_{k+1} = max(Wa_k V_k - b_k V_{k-1}, E),     S = V - E

i.e. the per-iteration bias add AND the relu collapse into a single DVE
tensor_tensor(max) against the fixed threshold tile E (computed in the
prologue as (-A(AtA)^-1).T @ X -- same cost as computing AtX).

Schedule: 12 Chebyshev ramp updates (exact [mu, L] from the host
eigendecomposition of AtA) then constant heavy-ball at the optimum
(alpha backed off 5% from the lambda=L stability edge), 23 updates total:
18 in bf16, then 5 fp32r polish updates on f32 states that contract the
bf16-phase state-rounding noise.  The phase transition is seamless: each
matmul picks the weight flavor matching its state operand's dtype.

Weight quantization uses per-entry SIGMA-DELTA sequences (rho-discounted
error feedback, rho = the heavy-ball contraction rate) instead of random
dither: with only ~7-9 updates per phase, the discounted average of
randomly-dithered variants leaves an O(1e-2) realization-dependent bias
in the fixed point; sigma-delta makes the effective weight unbiased and
the result deterministic.  Grids use the TRUE storage spacing (bf16: 7
explicit mantissa bits, fp32r: 11) so the stored tiles are exactly the
chosen values.  The prologue lhs is rounded with per-column error
feedback down the contraction axis, cancelling E's mean-field bias
against the row-iid X.  E stays f32 (it biases the fixed point 1:1).
Final rel err 1.128e-2 vs the 2e-2 gate, bit-reproduced by a host-side
replica of the exact tile values.

All inputs are host-packed into five [128, *] tensors whose partition
lines are single contiguous DMA descriptors; X rides both HW DMA queues
(SP+Act), trailing-phase weights follow the X halves.
"""

import os
import sys

import numpy as np

for _p in ("/opt/trn_rl_repo", "/root/.axon_site/_ro/trn_rl_repo"):
    if os.path.isdir(_p) and _p not in sys.path:
        sys.path.append(_p)

import ml_dtypes
from contextlib import ExitStack

import concourse.bass as bass
import concourse.bacc as bacc
import concourse.tile as tile
from concourse import mybir
from concourse.bass_utils import run_bass_kernel_spmd

M, KD, N_FULL, N_CORES = 512, 32, 32768, 8
N_RAMP = 12          # Chebyshev ramp updates (per-iteration weights)
N_BF16 = 18          # total bf16-phase updates (incl. ramp)
N_POLISH = 5         # fp32r polish updates
TOTAL_UPD = N_BF16 + N_POLISH
N_CONST = N_BF16 + 1 - N_RAMP   # constant-phase bf16 updates (k=13..21)
SD_RHO = 0.856       # heavy-ball contraction: sigma-delta discount factor
NSTR = 4             # independent column streams in the loop
ALPHA_BACKOFF = 0.95

F32 = mybir.dt.float32
F32R = mybir.dt.float32r
BF16 = mybir.dt.bfloat16

LAST_RESULTS = None  # BassKernelResults of the most recent run (for test.py)


def build_program(ns: int):
    q = ns // 4          # free extent of the packed [128, q] layout
    qs = q // NSTR       # columns per stream
    nsl = q // 512       # 512-wide prologue slices
    assert ns % 2048 == 0 and nsl >= 1 and qs % 256 == 0

    nc = bacc.Bacc("TRN2", target_bir_lowering=False)

    n_ramp_tiles = N_RAMP + (N_RAMP - 1)   # w_1..12 then bid_2..12
    n_wrest = n_ramp_tiles + (N_CONST + 1)
    # every [128,*] dma_start costs ~128 descriptors x ~25ns of HW-DGE
    # queue time regardless of size, so everything is packed into FIVE
    # big host-side tensors: X halves, wfront (w_1+ppad), wrest
    # (wramp+wconst), w32.
    x_d = nc.dram_tensor("x", [128, 8 * (ns // 2)], BF16,
                         kind="ExternalInput")
    wfront_d = nc.dram_tensor("wfront", [128, 128 + 16 * 128], BF16,
                              kind="ExternalInput")
    wrest_d = nc.dram_tensor("wrest", [128, n_wrest * 128], BF16,
                             kind="ExternalInput")
    n_w32 = TOTAL_UPD - (N_BF16 + 1)       # updates whose cur-state is f32
    w32_d = nc.dram_tensor("w32", [128, (n_w32 + 1) * 128], F32,
                           kind="ExternalInput")
    out_d = nc.dram_tensor("s_out", [KD, ns], F32, kind="ExternalOutput")

    with ExitStack() as ctx:
        tc = ctx.enter_context(tile.TileContext(nc))
        persist = ctx.enter_context(tc.tile_pool(name="persist", bufs=1))
        xpool = ctx.enter_context(tc.tile_pool(name="xstage", bufs=1))
        psum = ctx.enter_context(tc.tile_pool(name="psum", bufs=2,
                                              space="PSUM"))

        # sync: wfront (warm-up + prologue lhs + w_1), then X half 1;
        # scalar: X half 2 immediately, then the trailing weights.
        wf_sb = persist.tile([128, 128 + 16 * 128], BF16)
        nc.sync.dma_start(wf_sb[:], wfront_d[:])
        xh = ns // 2
        # X in four block-aligned pieces, one tile per DMA, byte-balanced
        # across the ~112GB/s queues (wfront rides sync) with SMALL last
        # pieces so the tail prologue matmuls start as early as possible.
        # col ranges in units of 1024: P3 scalar [7,13) lands first, then
        # P1 sync [0,4), then P2 sync [4,7) / P4 scalar [13,16).
        pieces = [(7168, 13312, nc.scalar), (0, 4096, nc.sync),
                  (4096, 7168, nc.sync), (13312, 16384, nc.scalar)]
        xp = []
        for pi, (lo, hi, eng) in enumerate(pieces):
            t = xpool.tile([128, hi - lo], BF16, name=f"xp{pi}")
            eng.dma_start(t[:], x_d[:, lo:hi])
            xp.append((lo, hi, t))
        wrest_sb = persist.tile([128, n_wrest * 128], BF16)
        nc.scalar.dma_start(wrest_sb[:], wrest_d[:])
        w32_sb = persist.tile([128, (n_w32 + 1) * 128], F32R)
        nc.scalar.dma_start(w32_sb[:], w32_d[:].bitcast(F32R))

        def xs(col, width):     # resolve global x column -> tile slice
            for lo, hi, t in xp:
                if lo <= col and col + width <= hi:
                    return t[:, col - lo:col - lo + width]
            raise AssertionError(col)

        def ppl(i):         # prologue lhs block i of wfront
            return wf_sb[:, 128 * (1 + i):128 * (2 + i)]

        def wr(i):          # ramp tile i (w_1 rides wfront)
            return wf_sb[:, 0:128] if i == 0 else \
                wrest_sb[:, 128 * i:128 * (i + 1)]

        def wc(i):          # constant-phase tile i
            return wrest_sb[:, 128 * (n_ramp_tiles + i):
                            128 * (n_ramp_tiles + i + 1)]

        # PE warm-up (junk matmuls on wfront) releases the PE_HAM
        # throttle during the X-DMA window.
        warm = psum.tile([128, 512], F32, name="warm", tag="pt1")
        for wi in range(24):
            nc.tensor.matmul(warm[:, 0:128], ppl(0), ppl(0),
                             start=(wi == 0), stop=(wi == 23))

        e_sb = persist.tile([128, q], F32)
        v_st = [[persist.tile([128, qs], BF16, name=f"v{i}_{j}")
                 for j in range(NSTR)] for i in range(3)]
        v32 = [[persist.tile([128, qs], F32R, name=f"v32_{i}_{j}")
                for j in range(NSTR)] for i in range(3)]
        sout = persist.tile([128, q], F32)

        # ---- prologue: E = (-A(AtA)^-1).T @ X in packed layout ----
        # emit (c,g) units in X-piece arrival order (PSUM accumulation
        # order is free); unit (c,g) covers global x cols
        # [(2c+h)*2048 + (g%2)*1024, +1024), h = g//2
        pbs = [psum.tile([128, 512], F32, name=f"pb{s}", tag=f"pt{2 * s}")
               for s in range(nsl)]
        n_acc = [0] * nsl
        units = []
        for c in range(4):
            for g in range(4):
                base = (2 * c + g // 2) * 2048 + (g % 2) * 1024
                pidx = next(i for i, (lo, hi, _) in enumerate(xp)
                            if lo <= base and base + 1024 <= hi)
                units.append((pidx, c, g, base))
        units.sort()
        for _, c, g, base in units:
            lhs = ppl(4 * c + g)
            for s in range(nsl):
                i_acc = n_acc[s]
                n_acc[s] = i_acc + 1
                nc.tensor.matmul(
                    pbs[s][:], lhs, xs(base + 512 * s, 512),
                    start=(i_acc == 0), stop=(i_acc == 15))
        spp = 512 // qs  # streams per prologue slice
        for s in range(nsl):
            nc.scalar.copy(e_sb[:, 512 * s:512 * (s + 1)], pbs[s][:])
            for jj in range(spp):
                j = spp * s + jj
                src = pbs[s][:, qs * jj:qs * (jj + 1)]
                # V1 = relu(E) straight from PSUM; Act also has the E
                # copies, so it only takes one stream
                if j == 1:
                    nc.scalar.activation(v_st[1][j][:], src,
                                         mybir.ActivationFunctionType.Relu)
                else:
                    nc.vector.tensor_scalar_max(v_st[1][j][:], src, 0.0)

        # ---- loop: update k computes V_{k+1} from V_k, V_{k-1} ----
        # state dtype: V_j is bf16 for j <= N_BF16+1, f32 after
        def vt(jgen, j):
            return (v_st if jgen <= N_BF16 + 1 else v32)[jgen % 3][j]

        for k in range(1, TOTAL_UPD + 1):
            last = (k == TOTAL_UPD)
            # cur-operand weight flavor (matches dtype of V_k)
            if k <= N_RAMP:
                wcur = wr(k - 1)
            elif k <= N_BF16 + 1:
                wcur = wc(k - (N_RAMP + 1))
            else:
                i = k - (N_BF16 + 2)
                wcur = w32_sb[:, 128 * i:128 * (i + 1)]
            # prev-operand flavor (matches dtype of V_{k-1}); k=1: b_1=0
            if k == 1:
                wprev = None
            elif k <= N_RAMP:
                wprev = wr(N_RAMP + k - 2)          # bid_k ramp tile
            elif k <= N_BF16 + 2:
                wprev = wc(N_CONST)
            else:
                wprev = w32_sb[:, 128 * n_w32:128 * (n_w32 + 1)]

            for j in range(NSTR):
                cur = vt(k, j) if k > 1 else v_st[1][j]
                prev = vt(k - 1, j) if k > 1 else None
                ptb = psum.tile([128, 512], F32, name=f"pt{k}_{j}",
                                tag=f"pt{j}")
                pt = ptb[:, 0:qs]
                esl = e_sb[:, qs * j:qs * (j + 1)]
                if k == 1:
                    nc.tensor.matmul(pt, wcur, cur[:],
                                     start=True, stop=True)
                else:
                    # -b*V_{k-1} first: never waits on the newest state
                    nc.tensor.matmul(pt, wprev, prev[:],
                                     start=True, stop=False)
                    nc.tensor.matmul(pt, wcur, cur[:],
                                     start=False, stop=True)
                if last:
                    # S = max(P,E)-E = relu(P-E): sub in PSUM, relu on Act
                    nc.vector.tensor_tensor(pt, pt, esl,
                                            op=mybir.AluOpType.subtract)
                    nc.scalar.activation(sout[:, qs * j:qs * (j + 1)], pt,
                                         mybir.ActivationFunctionType.Relu)
                else:
                    nc.vector.tensor_tensor(vt(k + 1, j)[:], pt, esl,
                                            op=mybir.AluOpType.max)

        for g in range(4):
            (nc.sync if g % 2 == 0 else nc.scalar).dma_start(
                out_d[:, g * q:(g + 1) * q], sout[32 * g:32 * (g + 1), :])

    nc.finalize()
    return nc


def _bf16(x):
    return np.asarray(x, dtype=np.float32).astype(ml_dtypes.bfloat16)


def _bf16_val(x):
    return float(np.float32(ml_dtypes.bfloat16(np.float32(x))))


def _round11(x):
    u = np.ascontiguousarray(np.asarray(x, dtype=np.float32)).view(np.uint32)
    u = ((u + np.uint32(1 << 11)) >> np.uint32(12)) << np.uint32(12)
    return u.view(np.float32)


def _dither(Mx, n, nbits):
    """n reduced-precision matrices whose per-entry mean ~= Mx."""
    M64 = np.asarray(Mx, dtype=np.float64)
    hi = (_bf16(M64) if nbits == 8 else _round11(M64)).astype(np.float64)
    ulp = 2.0 ** (np.floor(np.log2(np.abs(M64) + 1e-300)) - nbits)
    flo = np.where(hi > M64, hi - ulp, hi)
    fhi = flo + ulp
    frac = np.clip((M64 - flo) / ulp, 0, 1)
    cnt = np.rint(frac * n).astype(int)
    return [np.where(i < cnt, fhi, flo).astype(np.float32) for i in range(n)]


def _grid(M64, nbits):
    """neighboring representable values; nbits = EXPLICIT mantissa bits
    (bf16: 7, fp32r: 11) so the grid matches the storage dtype exactly."""
    hi = (_bf16(M64) if nbits == 7 else _round11(M64)).astype(np.float64)
    ulp = 2.0 ** (np.floor(np.log2(np.abs(M64) + 1e-300)) - nbits)
    flo = np.where(hi > M64, hi - ulp, hi)
    return flo, flo + ulp


def _sigma_delta(Mx, n, rho, nbits):
    """n rounded copies of Mx whose rho-discounted average is unbiased:
    per-entry error-feedback (sigma-delta) choice between the two
    neighboring representable values."""
    M64 = np.asarray(Mx, dtype=np.float64)
    flo, fhi = _grid(M64, nbits)
    D = np.zeros_like(M64)
    seq = []
    for _ in range(n):
        e_lo = rho * D + (flo - M64)
        e_hi = rho * D + (fhi - M64)
        Wq = np.where(np.abs(e_lo) <= np.abs(e_hi), flo, fhi)
        D = rho * D + (Wq - M64)
        seq.append(Wq.astype(np.float32))
    return seq


def _blockdiag(Mt, dtype):
    """lhsT tile: 4-group block-diagonal of Mt (already transposed)."""
    out = np.zeros((128, 128), dtype=dtype)
    for g in range(4):
        out[32 * g:32 * (g + 1), 32 * g:32 * (g + 1)] = Mt
    return out


def _cheby_params(L, mu, n_ramp, n_total):
    d = (L + mu) / 2.0
    cc = (L - mu) / 2.0
    al = [0.0] * (n_total + 1)
    be = [0.0] * (n_total + 1)
    w_prev = 0.0
    for k in range(1, n_total + 1):
        w = 1.0 / d if k == 1 else 1.0 / (d - cc * cc / 4.0 * w_prev)
        be[k] = (cc / 2.0) ** 2 * w_prev * w if k > 1 else 0.0
        al[k] = w
        w_prev = w
    kap = L / mu
    aH = ALPHA_BACKOFF * 4.0 / (np.sqrt(L) + np.sqrt(mu)) ** 2
    bH = ((np.sqrt(kap) - 1.0) / (np.sqrt(kap) + 1.0)) ** 2
    for k in range(n_ramp + 1, n_total + 1):
        al[k] = aH
        be[k] = bH
    return al, be


def host_prep(A: np.ndarray):
    A64 = np.asarray(A, dtype=np.float64)
    AtA = A64.T @ A64
    ev = np.linalg.eigvalsh(AtA)
    L, mu = ev[-1], ev[0]
    I = np.eye(KD)
    al, be = _cheby_params(L, mu, N_RAMP, TOTAL_UPD)

    # ramp tiles: w_1..N_RAMP (whole Wa_k, nearest-bf16), bid_2..N_RAMP
    n_ramp_tiles = N_RAMP + (N_RAMP - 1)
    wramp = np.zeros((128, n_ramp_tiles * 128), dtype=ml_dtypes.bfloat16)
    for k in range(1, N_RAMP + 1):
        bq = _bf16_val(be[k])
        Wa = (1.0 + bq) * I - al[k] * AtA
        wramp[:, 128 * (k - 1):128 * k] = _blockdiag(
            _bf16(Wa.T), ml_dtypes.bfloat16)
        if k >= 2:
            bid = np.zeros((32, 32), dtype=np.float64)
            np.fill_diagonal(bid, -bq)
            wramp[:, 128 * (N_RAMP + k - 2):128 * (N_RAMP + k - 1)] = \
                _blockdiag(_bf16(bid), ml_dtypes.bfloat16)

    # constant phase: one bf16-exact beta everywhere, sigma-delta
    # (rho-discounted per-entry error feedback) rounding sequences so the
    # effective weight seen by the fixed point is unbiased even over few
    # iterations (random dither left a ~1e-2 realization lottery).
    aH, bH = al[N_RAMP + 1], be[N_RAMP + 1]
    bHq = _bf16_val(bH)
    WaH = (1.0 + bHq) * I - aH * AtA
    n_w32 = TOTAL_UPD - (N_BF16 + 1)
    wconst = np.zeros((128, (N_CONST + 1) * 128), dtype=ml_dtypes.bfloat16)
    for i, v in enumerate(_sigma_delta(WaH.T, N_CONST, SD_RHO, 7)):
        wconst[:, 128 * i:128 * (i + 1)] = _blockdiag(v, ml_dtypes.bfloat16)
    bidH = np.diag([-bHq] * KD)
    wconst[:, 128 * N_CONST:] = _blockdiag(
        _bf16(bidH), ml_dtypes.bfloat16)

    # fp32r phase: same beta (bf16 value is f32- and fp32r-exact)
    w32 = np.zeros((128, (n_w32 + 1) * 128), dtype=np.float32)
    for i, v in enumerate(_sigma_delta(WaH.T, n_w32, SD_RHO, 11)):
        w32[:, 128 * i:128 * (i + 1)] = _blockdiag(v, np.float32)
    w32[:, 128 * n_w32:] = _blockdiag(
        np.diag([-bHq] * KD).astype(np.float32), np.float32)

    # prologue lhs: -A(AtA)^-1, bf16-rounded with per-column error
    # feedback DOWN THE ROW (contraction) AXIS: since E = Pq.T @ X and
    # X ~ U[0,1) is row-iid, keeping each column's running rounding
    # residual near zero cancels the mean-field bias of E.
    P = A64 @ np.linalg.inv(AtA)
    Pm = -P
    flo, fhi = _grid(Pm, 7)
    Pq = np.zeros_like(Pm)
    D = np.zeros(Pm.shape[1])
    for m in range(Pm.shape[0]):
        e_lo = D + (flo[m] - Pm[m])
        e_hi = D + (fhi[m] - Pm[m])
        take_lo = np.abs(e_lo) <= np.abs(e_hi)
        Pq[m] = np.where(take_lo, flo[m], fhi[m])
        D = np.where(take_lo, e_lo, e_hi)
    ppad = np.zeros((128, 16 * 128), dtype=ml_dtypes.bfloat16)
    for cch in range(4):
        pv = Pq[128 * cch:128 * (cch + 1), :]
        for g in range(4):
            ppad[:, 128 * (4 * cch + g) + 32 * g:
                 128 * (4 * cch + g) + 32 * (g + 1)] = _bf16(pv)

    # merge into the minimal-DMA layouts: wfront = [w_1 | ppad],
    # wrest = [wramp | wconst]
    wfront = np.concatenate([wramp[:, 0:128], ppad], axis=1)
    wrest = np.concatenate([wramp, wconst], axis=1)
    return wfront, wrest, w32


_PROGRAM_CACHE = {}


def _get_program(ns):
    if ns not in _PROGRAM_CACHE:
        _PROGRAM_CACHE[ns] = build_program(ns)
    return _PROGRAM_CACHE[ns]


def kernel(X: np.ndarray, A: np.ndarray) -> np.ndarray:
    global LAST_RESULTS
    X = np.ascontiguousarray(np.asarray(X, dtype=np.float32))
    A = np.ascontiguousarray(np.asarray(A, dtype=np.float32))
    assert X.shape == (M, N_FULL) and A.shape == (M, KD)

    ns = N_FULL // N_CORES
    wfront, wrest, w32 = host_prep(A)
    nc = _get_program(ns)

    Xb = _bf16(X)
    in_maps = []
    for c in range(N_CORES):
        # pack the shard so each partition line is one contiguous
        # descriptor: x[p, (2c+h)*2048 + j] = Xb[128c+p, 2048h+j]
        xs = np.asarray(Xb[:, c * ns:(c + 1) * ns])
        xp = np.ascontiguousarray(
            xs.reshape(4, 128, 2, ns // 2).transpose(1, 0, 2, 3)
            .reshape(128, 4 * ns))
        in_maps.append({
            "x": xp,
            "wfront": wfront,
            "wrest": wrest,
            "w32": w32,
        })

    res = run_bass_kernel_spmd(nc, in_maps, core_ids=list(range(N_CORES)))
    LAST_RESULTS = res
    S = np.concatenate([res.results[c]["s_out"] for c in range(N_CORES)],
                       axis=1)
    return np.ascontiguousarray(S.astype(np.float32))


# revision 47
# speedup vs baseline: 1.1667x; 1.0048x over previous
"""Trainium2 Bass kernel for nn_LsqNonneg: batched NNLS.

Algorithm: projected Chebyshev/heavy-ball on the V-substitution of the NNLS
KKT iteration.  With Wa_k = (1+b_k)I - a_k*AtA, the S-iteration

    S_{k+1} = relu(Wa_k S_k - b_k S_{k-1} + a_k AtX)

becomes, under V := S - Sunc with Sunc = (AtA)^-1 AtX  (E := -Sunc):

    V_{k+1} = max(Wa_k V_k - b_k V_{k-1}, E),     S = V - E

i.e. the per-iteration bias add AND the relu collapse into a single DVE
tensor_tensor(max) against the fixed threshold tile E (computed in the
prologue as (-A(AtA)^-1).T @ X -- same cost as computing AtX).

Schedule: 12 Chebyshev ramp updates (exact [mu, L] from the host
eigendecomposition of AtA) then constant heavy-ball at the optimum
(alpha backed off 5% from the lambda=L stability edge), 23 updates total:
18 in bf16, then 5 fp32r polish updates on f32 states that contract the
bf16-phase state-rounding noise.  The phase transition is seamless: each
matmul picks the weight flavor matching its state operand's dtype.

Weight quantization uses per-entry SIGMA-DELTA sequences (rho-discounted
error feedback, rho = the heavy-ball contraction rate) instead of random
dither: with only ~7-9 updates per phase, the discounted average of
randomly-dithered variants leaves an O(1e-2) realization-dependent bias
in the fixed point; sigma-delta makes the effective weight unbiased and
the result deterministic.  Grids use the TRUE storage spacing (bf16: 7
explicit mantissa bits, fp32r: 11) so the stored tiles are exactly the
chosen values.  The prologue lhs is rounded with per-column error
feedback down the contraction axis, cancelling E's mean-field bias
against the row-iid X.  E stays f32 (it biases the fixed point 1:1).
Final rel err 1.128e-2 vs the 2e-2 gate, bit-reproduced by a host-side
replica of the exact tile values.

All inputs are host-packed into five [128, *] tensors whose partition
lines are single contiguous DMA descriptors; X rides both HW DMA queues
(SP+Act), trailing-phase weights follow the X halves.
"""

import os
import sys

import numpy as np

for _p in ("/opt/trn_rl_repo", "/root/.axon_site/_ro/trn_rl_repo"):
    if os.path.isdir(_p) and _p not in sys.path:
        sys.path.append(_p)

import ml_dtypes
from contextlib import ExitStack

import concourse.bass as bass
import concourse.bacc as bacc
import concourse.tile as tile
from concourse import mybir
from concourse.bass_utils import run_bass_kernel_spmd

M, KD, N_FULL, N_CORES = 512, 32, 32768, 8
N_RAMP = 12          # Chebyshev ramp updates (per-iteration weights)
N_BF16 = 18          # total bf16-phase updates (incl. ramp)
N_POLISH = 4         # fp32r polish updates
TOTAL_UPD = N_BF16 + N_POLISH
N_CONST = N_BF16 + 1 - N_RAMP   # constant-phase bf16 updates (k=13..21)
SD_RHO = 0.856       # heavy-ball contraction: sigma-delta discount factor
NSTR = 4             # independent column streams in the loop
ALPHA_BACKOFF = 0.95

F32 = mybir.dt.float32
F32R = mybir.dt.float32r
BF16 = mybir.dt.bfloat16

LAST_RESULTS = None  # BassKernelResults of the most recent run (for test.py)


def build_program(ns: int):
    q = ns // 4          # free extent of the packed [128, q] layout
    qs = q // NSTR       # columns per stream
    nsl = q // 512       # 512-wide prologue slices
    assert ns % 2048 == 0 and nsl >= 1 and qs % 256 == 0

    nc = bacc.Bacc("TRN2", target_bir_lowering=False)

    n_ramp_tiles = N_RAMP + (N_RAMP - 1)   # w_1..12 then bid_2..12
    n_wrest = n_ramp_tiles + (N_CONST + 1)
    # every [128,*] dma_start costs ~128 descriptors x ~25ns of HW-DGE
    # queue time regardless of size, so everything is packed into FIVE
    # big host-side tensors: X halves, wfront (w_1+ppad), wrest
    # (wramp+wconst), w32.
    x_d = nc.dram_tensor("x", [128, 8 * (ns // 2)], BF16,
                         kind="ExternalInput")
    wfront_d = nc.dram_tensor("wfront", [128, 128 + 16 * 128], BF16,
                              kind="ExternalInput")
    wrest_d = nc.dram_tensor("wrest", [128, n_wrest * 128], BF16,
                             kind="ExternalInput")
    n_w32 = TOTAL_UPD - (N_BF16 + 1)       # updates whose cur-state is f32
    w32_d = nc.dram_tensor("w32", [128, (n_w32 + 1) * 128], F32,
                           kind="ExternalInput")
    out_d = nc.dram_tensor("s_out", [KD, ns], F32, kind="ExternalOutput")

    with ExitStack() as ctx:
        tc = ctx.enter_context(tile.TileContext(nc))
        persist = ctx.enter_context(tc.tile_pool(name="persist", bufs=1))
        xpool = ctx.enter_context(tc.tile_pool(name="xstage", bufs=1))
        psum = ctx.enter_context(tc.tile_pool(name="psum", bufs=2,
                                              space="PSUM"))

        # sync: wfront (warm-up + prologue lhs + w_1), then X half 1;
        # scalar: X half 2 immediately, then the trailing weights.
        wf_sb = persist.tile([128, 128 + 16 * 128], BF16)
        nc.sync.dma_start(wf_sb[:], wfront_d[:])
        xh = ns // 2
        # X in four block-aligned pieces, one tile per DMA, byte-balanced
        # across the ~112GB/s queues (wfront rides sync) with SMALL last
        # pieces so the tail prologue matmuls start as early as possible.
        # col ranges in units of 1024: P3 scalar [7,13) lands first, then
        # P1 sync [0,4), then P2 sync [4,7) / P4 scalar [13,16).
        pieces = [(7168, 13312, nc.scalar), (0, 4096, nc.sync),
                  (4096, 7168, nc.sync), (13312, 16384, nc.scalar)]
        xp = []
        for pi, (lo, hi, eng) in enumerate(pieces):
            t = xpool.tile([128, hi - lo], BF16, name=f"xp{pi}")
            eng.dma_start(t[:], x_d[:, lo:hi])
            xp.append((lo, hi, t))
        wrest_sb = persist.tile([128, n_wrest * 128], BF16)
        nc.scalar.dma_start(wrest_sb[:], wrest_d[:])
        w32_sb = persist.tile([128, (n_w32 + 1) * 128], F32R)
        nc.scalar.dma_start(w32_sb[:], w32_d[:].bitcast(F32R))

        def xs(col, width):     # resolve global x column -> tile slice
            for lo, hi, t in xp:
                if lo <= col and col + width <= hi:
                    return t[:, col - lo:col - lo + width]
            raise AssertionError(col)

        def ppl(i):         # prologue lhs block i of wfront
            return wf_sb[:, 128 * (1 + i):128 * (2 + i)]

        def wr(i):          # ramp tile i (w_1 rides wfront)
            return wf_sb[:, 0:128] if i == 0 else \
                wrest_sb[:, 128 * i:128 * (i + 1)]

        def wc(i):          # constant-phase tile i
            return wrest_sb[:, 128 * (n_ramp_tiles + i):
                            128 * (n_ramp_tiles + i + 1)]

        # PE warm-up (junk matmuls on wfront) releases the PE_HAM
        # throttle during the X-DMA window.
        warm = psum.tile([128, 512], F32, name="warm", tag="pt1")
        for wi in range(24):
            nc.tensor.matmul(warm[:, 0:128], ppl(0), ppl(0),
                             start=(wi == 0), stop=(wi == 23))

        e_sb = persist.tile([128, q], F32)
        v_st = [[persist.tile([128, qs], BF16, name=f"v{i}_{j}")
                 for j in range(NSTR)] for i in range(3)]
        v32 = [[persist.tile([128, qs], F32R, name=f"v32_{i}_{j}")
                for j in range(NSTR)] for i in range(3)]
        sout = persist.tile([128, q], F32)

        # ---- prologue: E = (-A(AtA)^-1).T @ X in packed layout ----
        # emit (c,g) units in X-piece arrival order (PSUM accumulation
        # order is free); unit (c,g) covers global x cols
        # [(2c+h)*2048 + (g%2)*1024, +1024), h = g//2
        pbs = [psum.tile([128, 512], F32, name=f"pb{s}", tag=f"pt{2 * s}")
               for s in range(nsl)]
        n_acc = [0] * nsl
        units = []
        for c in range(4):
            for g in range(4):
                base = (2 * c + g // 2) * 2048 + (g % 2) * 1024
                pidx = next(i for i, (lo, hi, _) in enumerate(xp)
                            if lo <= base and base + 1024 <= hi)
                units.append((pidx, c, g, base))
        units.sort()
        for _, c, g, base in units:
            lhs = ppl(4 * c + g)
            for s in range(nsl):
                i_acc = n_acc[s]
                n_acc[s] = i_acc + 1
                nc.tensor.matmul(
                    pbs[s][:], lhs, xs(base + 512 * s, 512),
                    start=(i_acc == 0), stop=(i_acc == 15))
        spp = 512 // qs  # streams per prologue slice
        for s in range(nsl):
            nc.scalar.copy(e_sb[:, 512 * s:512 * (s + 1)], pbs[s][:])
            for jj in range(spp):
                j = spp * s + jj
                src = pbs[s][:, qs * jj:qs * (jj + 1)]
                # V1 = relu(E) straight from PSUM; Act also has the E
                # copies, so it only takes one stream
                if j == 1:
                    nc.scalar.activation(v_st[1][j][:], src,
                                         mybir.ActivationFunctionType.Relu)
                else:
                    nc.vector.tensor_scalar_max(v_st[1][j][:], src, 0.0)

        # ---- loop: update k computes V_{k+1} from V_k, V_{k-1} ----
        # state dtype: V_j is bf16 for j <= N_BF16+1, f32 after
        def vt(jgen, j):
            return (v_st if jgen <= N_BF16 + 1 else v32)[jgen % 3][j]

        for k in range(1, TOTAL_UPD + 1):
            last = (k == TOTAL_UPD)
            # cur-operand weight flavor (matches dtype of V_k)
            if k <= N_RAMP:
                wcur = wr(k - 1)
            elif k <= N_BF16 + 1:
                wcur = wc(k - (N_RAMP + 1))
            else:
                i = k - (N_BF16 + 2)
                wcur = w32_sb[:, 128 * i:128 * (i + 1)]
            # prev-operand flavor (matches dtype of V_{k-1}); k=1: b_1=0
            if k == 1:
                wprev = None
            elif k <= N_RAMP:
                wprev = wr(N_RAMP + k - 2)          # bid_k ramp tile
            elif k <= N_BF16 + 2:
                wprev = wc(N_CONST)
            else:
                wprev = w32_sb[:, 128 * n_w32:128 * (n_w32 + 1)]

            for j in range(NSTR):
                cur = vt(k, j) if k > 1 else v_st[1][j]
                prev = vt(k - 1, j) if k > 1 else None
                ptb = psum.tile([128, 512], F32, name=f"pt{k}_{j}",
                                tag=f"pt{j}")
                pt = ptb[:, 0:qs]
                esl = e_sb[:, qs * j:qs * (j + 1)]
                if k == 1:
                    nc.tensor.matmul(pt, wcur, cur[:],
                                     start=True, stop=True)
                else:
                    # -b*V_{k-1} first: never waits on the newest state
                    nc.tensor.matmul(pt, wprev, prev[:],
                                     start=True, stop=False)
                    nc.tensor.matmul(pt, wcur, cur[:],
                                     start=False, stop=True)
                if last:
                    # S = max(P,E)-E = relu(P-E): sub in PSUM, relu on Act
                    nc.vector.tensor_tensor(pt, pt, esl,
                                            op=mybir.AluOpType.subtract)
                    nc.scalar.activation(sout[:, qs * j:qs * (j + 1)], pt,
                                         mybir.ActivationFunctionType.Relu)
                else:
                    nc.vector.tensor_tensor(vt(k + 1, j)[:], pt, esl,
                                            op=mybir.AluOpType.max)

        for g in range(4):
            (nc.sync if g % 2 == 0 else nc.scalar).dma_start(
                out_d[:, g * q:(g + 1) * q], sout[32 * g:32 * (g + 1), :])

    nc.finalize()
    return nc


def _bf16(x):
    return np.asarray(x, dtype=np.float32).astype(ml_dtypes.bfloat16)


def _bf16_val(x):
    return float(np.float32(ml_dtypes.bfloat16(np.float32(x))))


def _round11(x):
    u = np.ascontiguousarray(np.asarray(x, dtype=np.float32)).view(np.uint32)
    u = ((u + np.uint32(1 << 11)) >> np.uint32(12)) << np.uint32(12)
    return u.view(np.float32)


def _dither(Mx, n, nbits):
    """n reduced-precision matrices whose per-entry mean ~= Mx."""
    M64 = np.asarray(Mx, dtype=np.float64)
    hi = (_bf16(M64) if nbits == 8 else _round11(M64)).astype(np.float64)
    ulp = 2.0 ** (np.floor(np.log2(np.abs(M64) + 1e-300)) - nbits)
    flo = np.where(hi > M64, hi - ulp, hi)
    fhi = flo + ulp
    frac = np.clip((M64 - flo) / ulp, 0, 1)
    cnt = np.rint(frac * n).astype(int)
    return [np.where(i < cnt, fhi, flo).astype(np.float32) for i in range(n)]


def _grid(M64, nbits):
    """neighboring representable values; nbits = EXPLICIT mantissa bits
    (bf16: 7, fp32r: 11) so the grid matches the storage dtype exactly."""
    hi = (_bf16(M64) if nbits == 7 else _round11(M64)).astype(np.float64)
    ulp = 2.0 ** (np.floor(np.log2(np.abs(M64) + 1e-300)) - nbits)
    flo = np.where(hi > M64, hi - ulp, hi)
    return flo, flo + ulp


def _sigma_delta(Mx, n, rho, nbits):
    """n rounded copies of Mx whose rho-discounted average is unbiased:
    per-entry error-feedback (sigma-delta) choice between the two
    neighboring representable values."""
    M64 = np.asarray(Mx, dtype=np.float64)
    flo, fhi = _grid(M64, nbits)
    D = np.zeros_like(M64)
    seq = []
    for _ in range(n):
        e_lo = rho * D + (flo - M64)
        e_hi = rho * D + (fhi - M64)
        Wq = np.where(np.abs(e_lo) <= np.abs(e_hi), flo, fhi)
        D = rho * D + (Wq - M64)
        seq.append(Wq.astype(np.float32))
    return seq


def _blockdiag(Mt, dtype):
    """lhsT tile: 4-group block-diagonal of Mt (already transposed)."""
    out = np.zeros((128, 128), dtype=dtype)
    for g in range(4):
        out[32 * g:32 * (g + 1), 32 * g:32 * (g + 1)] = Mt
    return out


def _cheby_params(L, mu, n_ramp, n_total):
    d = (L + mu) / 2.0
    cc = (L - mu) / 2.0
    al = [0.0] * (n_total + 1)
    be = [0.0] * (n_total + 1)
    w_prev = 0.0
    for k in range(1, n_total + 1):
        w = 1.0 / d if k == 1 else 1.0 / (d - cc * cc / 4.0 * w_prev)
        be[k] = (cc / 2.0) ** 2 * w_prev * w if k > 1 else 0.0
        al[k] = w
        w_prev = w
    kap = L / mu
    aH = ALPHA_BACKOFF * 4.0 / (np.sqrt(L) + np.sqrt(mu)) ** 2
    bH = ((np.sqrt(kap) - 1.0) / (np.sqrt(kap) + 1.0)) ** 2
    for k in range(n_ramp + 1, n_total + 1):
        al[k] = aH
        be[k] = bH
    return al, be


def host_prep(A: np.ndarray):
    A64 = np.asarray(A, dtype=np.float64)
    AtA = A64.T @ A64
    ev = np.linalg.eigvalsh(AtA)
    L, mu = ev[-1], ev[0]
    I = np.eye(KD)
    al, be = _cheby_params(L, mu, N_RAMP, TOTAL_UPD)

    # ramp tiles: w_1..N_RAMP (whole Wa_k, nearest-bf16), bid_2..N_RAMP
    n_ramp_tiles = N_RAMP + (N_RAMP - 1)
    wramp = np.zeros((128, n_ramp_tiles * 128), dtype=ml_dtypes.bfloat16)
    for k in range(1, N_RAMP + 1):
        bq = _bf16_val(be[k])
        Wa = (1.0 + bq) * I - al[k] * AtA
        wramp[:, 128 * (k - 1):128 * k] = _blockdiag(
            _bf16(Wa.T), ml_dtypes.bfloat16)
        if k >= 2:
            bid = np.zeros((32, 32), dtype=np.float64)
            np.fill_diagonal(bid, -bq)
            wramp[:, 128 * (N_RAMP + k - 2):128 * (N_RAMP + k - 1)] = \
                _blockdiag(_bf16(bid), ml_dtypes.bfloat16)

    # constant phase: one bf16-exact beta everywhere, sigma-delta
    # (rho-discounted per-entry error feedback) rounding sequences so the
    # effective weight seen by the fixed point is unbiased even over few
    # iterations (random dither left a ~1e-2 realization lottery).
    aH, bH = al[N_RAMP + 1], be[N_RAMP + 1]
    bHq = _bf16_val(bH)
    WaH = (1.0 + bHq) * I - aH * AtA
    n_w32 = TOTAL_UPD - (N_BF16 + 1)
    wconst = np.zeros((128, (N_CONST + 1) * 128), dtype=ml_dtypes.bfloat16)
    for i, v in enumerate(_sigma_delta(WaH.T, N_CONST, SD_RHO, 7)):
        wconst[:, 128 * i:128 * (i + 1)] = _blockdiag(v, ml_dtypes.bfloat16)
    bidH = np.diag([-bHq] * KD)
    wconst[:, 128 * N_CONST:] = _blockdiag(
        _bf16(bidH), ml_dtypes.bfloat16)

    # fp32r phase: same beta (bf16 value is f32- and fp32r-exact)
    w32 = np.zeros((128, (n_w32 + 1) * 128), dtype=np.float32)
    for i, v in enumerate(_sigma_delta(WaH.T, n_w32, SD_RHO, 11)):
        w32[:, 128 * i:128 * (i + 1)] = _blockdiag(v, np.float32)
    w32[:, 128 * n_w32:] = _blockdiag(
        np.diag([-bHq] * KD).astype(np.float32), np.float32)

    # prologue lhs: -A(AtA)^-1, bf16-rounded with per-column error
    # feedback DOWN THE ROW (contraction) AXIS: since E = Pq.T @ X and
    # X ~ U[0,1) is row-iid, keeping each column's running rounding
    # residual near zero cancels the mean-field bias of E.
    P = A64 @ np.linalg.inv(AtA)
    Pm = -P
    flo, fhi = _grid(Pm, 7)
    Pq = np.zeros_like(Pm)
    D = np.zeros(Pm.shape[1])
    for m in range(Pm.shape[0]):
        e_lo = D + (flo[m] - Pm[m])
        e_hi = D + (fhi[m] - Pm[m])
        take_lo = np.abs(e_lo) <= np.abs(e_hi)
        Pq[m] = np.where(take_lo, flo[m], fhi[m])
        D = np.where(take_lo, e_lo, e_hi)
    ppad = np.zeros((128, 16 * 128), dtype=ml_dtypes.bfloat16)
    for cch in range(4):
        pv = Pq[128 * cch:128 * (cch + 1), :]
        for g in range(4):
            ppad[:, 128 * (4 * cch + g) + 32 * g:
                 128 * (4 * cch + g) + 32 * (g + 1)] = _bf16(pv)

    # merge into the minimal-DMA layouts: wfront = [w_1 | ppad],
    # wrest = [wramp | wconst]
    wfront = np.concatenate([wramp[:, 0:128], ppad], axis=1)
    wrest = np.concatenate([wramp, wconst], axis=1)
    return wfront, wrest, w32


_PROGRAM_CACHE = {}


def _get_program(ns):
    if ns not in _PROGRAM_CACHE:
        _PROGRAM_CACHE[ns] = build_program(ns)
    return _PROGRAM_CACHE[ns]


def kernel(X: np.ndarray, A: np.ndarray) -> np.ndarray:
    global LAST_RESULTS
    X = np.ascontiguousarray(np.asarray(X, dtype=np.float32))
    A = np.ascontiguousarray(np.asarray(A, dtype=np.float32))
    assert X.shape == (M, N_FULL) and A.shape == (M, KD)

    ns = N_FULL // N_CORES
    wfront, wrest, w32 = host_prep(A)
    nc = _get_program(ns)

    Xb = _bf16(X)
    in_maps = []
    for c in range(N_CORES):
        # pack the shard so each partition line is one contiguous
        # descriptor: x[p, (2c+h)*2048 + j] = Xb[128c+p, 2048h+j]
        xs = np.asarray(Xb[:, c * ns:(c + 1) * ns])
        xp = np.ascontiguousarray(
            xs.reshape(4, 128, 2, ns // 2).transpose(1, 0, 2, 3)
            .reshape(128, 4 * ns))
        in_maps.append({
            "x": xp,
            "wfront": wfront,
            "wrest": wrest,
            "w32": w32,
        })

    res = run_bass_kernel_spmd(nc, in_maps, core_ids=list(range(N_CORES)))
    LAST_RESULTS = res
    S = np.concatenate([res.results[c]["s_out"] for c in range(N_CORES)],
                       axis=1)
    return np.ascontiguousarray(S.astype(np.float32))


# revision 48
# speedup vs baseline: 1.1796x; 1.0111x over previous
"""Trainium2 Bass kernel for nn_LsqNonneg: batched NNLS.

Algorithm: projected Chebyshev/heavy-ball on the V-substitution of the NNLS
KKT iteration.  With Wa_k = (1+b_k)I - a_k*AtA, the S-iteration

    S_{k+1} = relu(Wa_k S_k - b_k S_{k-1} + a_k AtX)

becomes, under V := S - Sunc with Sunc = (AtA)^-1 AtX  (E := -Sunc):

    V_{k+1} = max(Wa_k V_k - b_k V_{k-1}, E),     S = V - E

i.e. the per-iteration bias add AND the relu collapse into a single DVE
tensor_tensor(max) against the fixed threshold tile E (computed in the
prologue as (-A(AtA)^-1).T @ X -- same cost as computing AtX).

Schedule: 12 Chebyshev ramp updates (exact [mu, L] from the host
eigendecomposition of AtA) then constant heavy-ball at the optimum
(alpha backed off 5% from the lambda=L stability edge), 23 updates total:
18 in bf16, then 5 fp32r polish updates on f32 states that contract the
bf16-phase state-rounding noise.  The phase transition is seamless: each
matmul picks the weight flavor matching its state operand's dtype.

Weight quantization uses per-entry SIGMA-DELTA sequences (rho-discounted
error feedback, rho = the heavy-ball contraction rate) instead of random
dither: with only ~7-9 updates per phase, the discounted average of
randomly-dithered variants leaves an O(1e-2) realization-dependent bias
in the fixed point; sigma-delta makes the effective weight unbiased and
the result deterministic.  Grids use the TRUE storage spacing (bf16: 7
explicit mantissa bits, fp32r: 11) so the stored tiles are exactly the
chosen values.  The prologue lhs is rounded with per-column error
feedback down the contraction axis, cancelling E's mean-field bias
against the row-iid X.  E stays f32 (it biases the fixed point 1:1).
Final rel err 1.128e-2 vs the 2e-2 gate, bit-reproduced by a host-side
replica of the exact tile values.

All inputs are host-packed into five [128, *] tensors whose partition
lines are single contiguous DMA descriptors; X rides both HW DMA queues
(SP+Act), trailing-phase weights follow the X halves.
"""

import os
import sys

import numpy as np

for _p in ("/opt/trn_rl_repo", "/root/.axon_site/_ro/trn_rl_repo"):
    if os.path.isdir(_p) and _p not in sys.path:
        sys.path.append(_p)

import ml_dtypes
from contextlib import ExitStack

import concourse.bass as bass
import concourse.bacc as bacc
import concourse.tile as tile
from concourse import mybir
from concourse.bass_utils import run_bass_kernel_spmd

M, KD, N_FULL, N_CORES = 512, 32, 32768, 8
N_RAMP = 12          # Chebyshev ramp updates (per-iteration weights)
N_BF16 = 18          # total bf16-phase updates (incl. ramp)
N_POLISH = 4         # fp32r polish updates
TOTAL_UPD = N_BF16 + N_POLISH
N_CONST = N_BF16 + 1 - N_RAMP   # constant-phase bf16 updates (k=13..21)
SD_RHO = 0.856       # heavy-ball contraction: sigma-delta discount factor
NSTR = 2             # independent column streams in the loop (512-wide
                     # ops halve the loop instruction count: the span is
                     # partly sequencer-paced, not just engine-paced)
ALPHA_BACKOFF = 0.95

F32 = mybir.dt.float32
F32R = mybir.dt.float32r
BF16 = mybir.dt.bfloat16

LAST_RESULTS = None  # BassKernelResults of the most recent run (for test.py)


def build_program(ns: int):
    q = ns // 4          # free extent of the packed [128, q] layout
    qs = q // NSTR       # columns per stream
    nsl = q // 512       # 512-wide prologue slices
    assert ns % 2048 == 0 and nsl >= 1 and qs % 256 == 0

    nc = bacc.Bacc("TRN2", target_bir_lowering=False)

    n_ramp_tiles = N_RAMP + (N_RAMP - 1)   # w_1..12 then bid_2..12
    n_wrest = n_ramp_tiles + (N_CONST + 1)
    # every [128,*] dma_start costs ~128 descriptors x ~25ns of HW-DGE
    # queue time regardless of size, so everything is packed into FIVE
    # big host-side tensors: X halves, wfront (w_1+ppad), wrest
    # (wramp+wconst), w32.
    x_d = nc.dram_tensor("x", [128, 8 * (ns // 2)], BF16,
                         kind="ExternalInput")
    wfront_d = nc.dram_tensor("wfront", [128, 128 + 16 * 128], BF16,
                              kind="ExternalInput")
    wrest_d = nc.dram_tensor("wrest", [128, n_wrest * 128], BF16,
                             kind="ExternalInput")
    n_w32 = TOTAL_UPD - (N_BF16 + 1)       # updates whose cur-state is f32
    w32_d = nc.dram_tensor("w32", [128, (n_w32 + 1) * 128], F32,
                           kind="ExternalInput")
    out_d = nc.dram_tensor("s_out", [KD, ns], F32, kind="ExternalOutput")

    with ExitStack() as ctx:
        tc = ctx.enter_context(tile.TileContext(nc))
        persist = ctx.enter_context(tc.tile_pool(name="persist", bufs=1))
        xpool = ctx.enter_context(tc.tile_pool(name="xstage", bufs=1))
        psum = ctx.enter_context(tc.tile_pool(name="psum", bufs=2,
                                              space="PSUM"))

        # sync: wfront (warm-up + prologue lhs + w_1), then X half 1;
        # scalar: X half 2 immediately, then the trailing weights.
        wf_sb = persist.tile([128, 128 + 16 * 128], BF16)
        nc.sync.dma_start(wf_sb[:], wfront_d[:])
        xh = ns // 2
        # X in four block-aligned pieces, one tile per DMA, byte-balanced
        # across the ~112GB/s queues (wfront rides sync) with SMALL last
        # pieces so the tail prologue matmuls start as early as possible.
        # col ranges in units of 1024: P3 scalar [7,13) lands first, then
        # P1 sync [0,4), then P2 sync [4,7) / P4 scalar [13,16).
        pieces = [(7168, 13312, nc.scalar), (0, 4096, nc.sync),
                  (4096, 7168, nc.sync), (13312, 16384, nc.scalar)]
        xp = []
        for pi, (lo, hi, eng) in enumerate(pieces):
            t = xpool.tile([128, hi - lo], BF16, name=f"xp{pi}")
            eng.dma_start(t[:], x_d[:, lo:hi])
            xp.append((lo, hi, t))
        wrest_sb = persist.tile([128, n_wrest * 128], BF16)
        nc.scalar.dma_start(wrest_sb[:], wrest_d[:])
        w32_sb = persist.tile([128, (n_w32 + 1) * 128], F32R)
        nc.scalar.dma_start(w32_sb[:], w32_d[:].bitcast(F32R))

        def xs(col, width):     # resolve global x column -> tile slice
            for lo, hi, t in xp:
                if lo <= col and col + width <= hi:
                    return t[:, col - lo:col - lo + width]
            raise AssertionError(col)

        def ppl(i):         # prologue lhs block i of wfront
            return wf_sb[:, 128 * (1 + i):128 * (2 + i)]

        def wr(i):          # ramp tile i (w_1 rides wfront)
            return wf_sb[:, 0:128] if i == 0 else \
                wrest_sb[:, 128 * i:128 * (i + 1)]

        def wc(i):          # constant-phase tile i
            return wrest_sb[:, 128 * (n_ramp_tiles + i):
                            128 * (n_ramp_tiles + i + 1)]

        # PE warm-up (junk matmuls on wfront) releases the PE_HAM
        # throttle during the X-DMA window.
        warm = psum.tile([128, 512], F32, name="warm", tag="pt1")
        for wi in range(24):
            nc.tensor.matmul(warm[:, 0:128], ppl(0), ppl(0),
                             start=(wi == 0), stop=(wi == 23))

        e_sb = persist.tile([128, q], F32)
        v_st = [[persist.tile([128, qs], BF16, name=f"v{i}_{j}")
                 for j in range(NSTR)] for i in range(3)]
        v32 = [[persist.tile([128, qs], F32R, name=f"v32_{i}_{j}")
                for j in range(NSTR)] for i in range(3)]
        sout = persist.tile([128, q], F32)

        # ---- prologue: E = (-A(AtA)^-1).T @ X in packed layout ----
        # emit (c,g) units in X-piece arrival order (PSUM accumulation
        # order is free); unit (c,g) covers global x cols
        # [(2c+h)*2048 + (g%2)*1024, +1024), h = g//2
        pbs = [psum.tile([128, 512], F32, name=f"pb{s}", tag=f"pt{2 * s}")
               for s in range(nsl)]
        n_acc = [0] * nsl
        units = []
        for c in range(4):
            for g in range(4):
                base = (2 * c + g // 2) * 2048 + (g % 2) * 1024
                pidx = next(i for i, (lo, hi, _) in enumerate(xp)
                            if lo <= base and base + 1024 <= hi)
                units.append((pidx, c, g, base))
        units.sort()
        for _, c, g, base in units:
            lhs = ppl(4 * c + g)
            for s in range(nsl):
                i_acc = n_acc[s]
                n_acc[s] = i_acc + 1
                nc.tensor.matmul(
                    pbs[s][:], lhs, xs(base + 512 * s, 512),
                    start=(i_acc == 0), stop=(i_acc == 15))
        spp = 512 // qs  # streams per prologue slice
        for s in range(nsl):
            nc.scalar.copy(e_sb[:, 512 * s:512 * (s + 1)], pbs[s][:])
            for jj in range(spp):
                j = spp * s + jj
                src = pbs[s][:, qs * jj:qs * (jj + 1)]
                # V1 = relu(E) straight from PSUM; Act also has the E
                # copies, so it only takes one stream
                if j == 1:
                    nc.scalar.activation(v_st[1][j][:], src,
                                         mybir.ActivationFunctionType.Relu)
                else:
                    nc.vector.tensor_scalar_max(v_st[1][j][:], src, 0.0)

        # ---- loop: update k computes V_{k+1} from V_k, V_{k-1} ----
        # state dtype: V_j is bf16 for j <= N_BF16+1, f32 after
        def vt(jgen, j):
            return (v_st if jgen <= N_BF16 + 1 else v32)[jgen % 3][j]

        for k in range(1, TOTAL_UPD + 1):
            last = (k == TOTAL_UPD)
            # cur-operand weight flavor (matches dtype of V_k)
            if k <= N_RAMP:
                wcur = wr(k - 1)
            elif k <= N_BF16 + 1:
                wcur = wc(k - (N_RAMP + 1))
            else:
                i = k - (N_BF16 + 2)
                wcur = w32_sb[:, 128 * i:128 * (i + 1)]
            # prev-operand flavor (matches dtype of V_{k-1}); k=1: b_1=0
            if k == 1:
                wprev = None
            elif k <= N_RAMP:
                wprev = wr(N_RAMP + k - 2)          # bid_k ramp tile
            elif k <= N_BF16 + 2:
                wprev = wc(N_CONST)
            else:
                wprev = w32_sb[:, 128 * n_w32:128 * (n_w32 + 1)]

            for j in range(NSTR):
                cur = vt(k, j) if k > 1 else v_st[1][j]
                prev = vt(k - 1, j) if k > 1 else None
                ptb = psum.tile([128, 512], F32, name=f"pt{k}_{j}",
                                tag=f"pt{j}")
                pt = ptb[:, 0:qs]
                esl = e_sb[:, qs * j:qs * (j + 1)]
                if k == 1:
                    nc.tensor.matmul(pt, wcur, cur[:],
                                     start=True, stop=True)
                else:
                    # -b*V_{k-1} first: never waits on the newest state
                    nc.tensor.matmul(pt, wprev, prev[:],
                                     start=True, stop=False)
                    nc.tensor.matmul(pt, wcur, cur[:],
                                     start=False, stop=True)
                if last:
                    # S = max(P,E)-E = relu(P-E): sub in PSUM, relu on Act
                    nc.vector.tensor_tensor(pt, pt, esl,
                                            op=mybir.AluOpType.subtract)
                    nc.scalar.activation(sout[:, qs * j:qs * (j + 1)], pt,
                                         mybir.ActivationFunctionType.Relu)
                else:
                    nc.vector.tensor_tensor(vt(k + 1, j)[:], pt, esl,
                                            op=mybir.AluOpType.max)

        for g in range(4):
            (nc.sync if g % 2 == 0 else nc.scalar).dma_start(
                out_d[:, g * q:(g + 1) * q], sout[32 * g:32 * (g + 1), :])

    nc.finalize()
    return nc


def _bf16(x):
    return np.asarray(x, dtype=np.float32).astype(ml_dtypes.bfloat16)


def _bf16_val(x):
    return float(np.float32(ml_dtypes.bfloat16(np.float32(x))))


def _round11(x):
    u = np.ascontiguousarray(np.asarray(x, dtype=np.float32)).view(np.uint32)
    u = ((u + np.uint32(1 << 11)) >> np.uint32(12)) << np.uint32(12)
    return u.view(np.float32)


def _dither(Mx, n, nbits):
    """n reduced-precision matrices whose per-entry mean ~= Mx."""
    M64 = np.asarray(Mx, dtype=np.float64)
    hi = (_bf16(M64) if nbits == 8 else _round11(M64)).astype(np.float64)
    ulp = 2.0 ** (np.floor(np.log2(np.abs(M64) + 1e-300)) - nbits)
    flo = np.where(hi > M64, hi - ulp, hi)
    fhi = flo + ulp
    frac = np.clip((M64 - flo) / ulp, 0, 1)
    cnt = np.rint(frac * n).astype(int)
    return [np.where(i < cnt, fhi, flo).astype(np.float32) for i in range(n)]


def _grid(M64, nbits):
    """neighboring representable values; nbits = EXPLICIT mantissa bits
    (bf16: 7, fp32r: 11) so the grid matches the storage dtype exactly."""
    hi = (_bf16(M64) if nbits == 7 else _round11(M64)).astype(np.float64)
    ulp = 2.0 ** (np.floor(np.log2(np.abs(M64) + 1e-300)) - nbits)
    flo = np.where(hi > M64, hi - ulp, hi)
    return flo, flo + ulp


def _sigma_delta(Mx, n, rho, nbits):
    """n rounded copies of Mx whose rho-discounted average is unbiased:
    per-entry error-feedback (sigma-delta) choice between the two
    neighboring representable values."""
    M64 = np.asarray(Mx, dtype=np.float64)
    flo, fhi = _grid(M64, nbits)
    D = np.zeros_like(M64)
    seq = []
    for _ in range(n):
        e_lo = rho * D + (flo - M64)
        e_hi = rho * D + (fhi - M64)
        Wq = np.where(np.abs(e_lo) <= np.abs(e_hi), flo, fhi)
        D = rho * D + (Wq - M64)
        seq.append(Wq.astype(np.float32))
    return seq


def _blockdiag(Mt, dtype):
    """lhsT tile: 4-group block-diagonal of Mt (already transposed)."""
    out = np.zeros((128, 128), dtype=dtype)
    for g in range(4):
        out[32 * g:32 * (g + 1), 32 * g:32 * (g + 1)] = Mt
    return out


def _cheby_params(L, mu, n_ramp, n_total):
    d = (L + mu) / 2.0
    cc = (L - mu) / 2.0
    al = [0.0] * (n_total + 1)
    be = [0.0] * (n_total + 1)
    w_prev = 0.0
    for k in range(1, n_total + 1):
        w = 1.0 / d if k == 1 else 1.0 / (d - cc * cc / 4.0 * w_prev)
        be[k] = (cc / 2.0) ** 2 * w_prev * w if k > 1 else 0.0
        al[k] = w
        w_prev = w
    kap = L / mu
    aH = ALPHA_BACKOFF * 4.0 / (np.sqrt(L) + np.sqrt(mu)) ** 2
    bH = ((np.sqrt(kap) - 1.0) / (np.sqrt(kap) + 1.0)) ** 2
    for k in range(n_ramp + 1, n_total + 1):
        al[k] = aH
        be[k] = bH
    return al, be


def host_prep(A: np.ndarray):
    A64 = np.asarray(A, dtype=np.float64)
    AtA = A64.T @ A64
    ev = np.linalg.eigvalsh(AtA)
    L, mu = ev[-1], ev[0]
    I = np.eye(KD)
    al, be = _cheby_params(L, mu, N_RAMP, TOTAL_UPD)

    # ramp tiles: w_1..N_RAMP (whole Wa_k, nearest-bf16), bid_2..N_RAMP
    n_ramp_tiles = N_RAMP + (N_RAMP - 1)
    wramp = np.zeros((128, n_ramp_tiles * 128), dtype=ml_dtypes.bfloat16)
    for k in range(1, N_RAMP + 1):
        bq = _bf16_val(be[k])
        Wa = (1.0 + bq) * I - al[k] * AtA
        wramp[:, 128 * (k - 1):128 * k] = _blockdiag(
            _bf16(Wa.T), ml_dtypes.bfloat16)
        if k >= 2:
            bid = np.zeros((32, 32), dtype=np.float64)
            np.fill_diagonal(bid, -bq)
            wramp[:, 128 * (N_RAMP + k - 2):128 * (N_RAMP + k - 1)] = \
                _blockdiag(_bf16(bid), ml_dtypes.bfloat16)

    # constant phase: one bf16-exact beta everywhere, sigma-delta
    # (rho-discounted per-entry error feedback) rounding sequences so the
    # effective weight seen by the fixed point is unbiased even over few
    # iterations (random dither left a ~1e-2 realization lottery).
    aH, bH = al[N_RAMP + 1], be[N_RAMP + 1]
    bHq = _bf16_val(bH)
    WaH = (1.0 + bHq) * I - aH * AtA
    n_w32 = TOTAL_UPD - (N_BF16 + 1)
    wconst = np.zeros((128, (N_CONST + 1) * 128), dtype=ml_dtypes.bfloat16)
    for i, v in enumerate(_sigma_delta(WaH.T, N_CONST, SD_RHO, 7)):
        wconst[:, 128 * i:128 * (i + 1)] = _blockdiag(v, ml_dtypes.bfloat16)
    bidH = np.diag([-bHq] * KD)
    wconst[:, 128 * N_CONST:] = _blockdiag(
        _bf16(bidH), ml_dtypes.bfloat16)

    # fp32r phase: same beta (bf16 value is f32- and fp32r-exact)
    w32 = np.zeros((128, (n_w32 + 1) * 128), dtype=np.float32)
    for i, v in enumerate(_sigma_delta(WaH.T, n_w32, SD_RHO, 11)):
        w32[:, 128 * i:128 * (i + 1)] = _blockdiag(v, np.float32)
    w32[:, 128 * n_w32:] = _blockdiag(
        np.diag([-bHq] * KD).astype(np.float32), np.float32)

    # prologue lhs: -A(AtA)^-1, bf16-rounded with per-column error
    # feedback DOWN THE ROW (contraction) AXIS: since E = Pq.T @ X and
    # X ~ U[0,1) is row-iid, keeping each column's running rounding
    # residual near zero cancels the mean-field bias of E.
    P = A64 @ np.linalg.inv(AtA)
    Pm = -P
    flo, fhi = _grid(Pm, 7)
    Pq = np.zeros_like(Pm)
    D = np.zeros(Pm.shape[1])
    for m in range(Pm.shape[0]):
        e_lo = D + (flo[m] - Pm[m])
        e_hi = D + (fhi[m] - Pm[m])
        take_lo = np.abs(e_lo) <= np.abs(e_hi)
        Pq[m] = np.where(take_lo, flo[m], fhi[m])
        D = np.where(take_lo, e_lo, e_hi)
    ppad = np.zeros((128, 16 * 128), dtype=ml_dtypes.bfloat16)
    for cch in range(4):
        pv = Pq[128 * cch:128 * (cch + 1), :]
        for g in range(4):
            ppad[:, 128 * (4 * cch + g) + 32 * g:
                 128 * (4 * cch + g) + 32 * (g + 1)] = _bf16(pv)

    # merge into the minimal-DMA layouts: wfront = [w_1 | ppad],
    # wrest = [wramp | wconst]
    wfront = np.concatenate([wramp[:, 0:128], ppad], axis=1)
    wrest = np.concatenate([wramp, wconst], axis=1)
    return wfront, wrest, w32


_PROGRAM_CACHE = {}


def _get_program(ns):
    if ns not in _PROGRAM_CACHE:
        _PROGRAM_CACHE[ns] = build_program(ns)
    return _PROGRAM_CACHE[ns]


def kernel(X: np.ndarray, A: np.ndarray) -> np.ndarray:
    global LAST_RESULTS
    X = np.ascontiguousarray(np.asarray(X, dtype=np.float32))
    A = np.ascontiguousarray(np.asarray(A, dtype=np.float32))
    assert X.shape == (M, N_FULL) and A.shape == (M, KD)

    ns = N_FULL // N_CORES
    wfront, wrest, w32 = host_prep(A)
    nc = _get_program(ns)

    Xb = _bf16(X)
    in_maps = []
    for c in range(N_CORES):
        # pack the shard so each partition line is one contiguous
        # descriptor: x[p, (2c+h)*2048 + j] = Xb[128c+p, 2048h+j]
        xs = np.asarray(Xb[:, c * ns:(c + 1) * ns])
        xp = np.ascontiguousarray(
            xs.reshape(4, 128, 2, ns // 2).transpose(1, 0, 2, 3)
            .reshape(128, 4 * ns))
        in_maps.append({
            "x": xp,
            "wfront": wfront,
            "wrest": wrest,
            "w32": w32,
        })

    res = run_bass_kernel_spmd(nc, in_maps, core_ids=list(range(N_CORES)))
    LAST_RESULTS = res
    S = np.concatenate([res.results[c]["s_out"] for c in range(N_CORES)],
                       axis=1)
    return np.ascontiguousarray(S.astype(np.float32))
